# revision 25
# baseline (speedup 1.0000x reference)
"""BoundaryAwareEncoder Trainium2 kernel.

Data-parallel over batch: 16 rows -> 8 cores x 2 rows. Activations kept
TRANSPOSED in SBUF (xT [d, S], d on partitions) so every linear uses the
natural weight layout as lhsT. Softmax runs with keys on partitions (scores
magnitude <= ~1.5 so no max-subtraction needed). Segment-mean is a one-hot
matmul built on-device from a cumsum (tensor_tensor_scan) of the merge
decisions. bf16 matmuls with fp32 PSUM accumulation throughout.
"""
import math
import numpy as np

P = 128
HD = [512, 512, 640, 768, 768]
NH = 8
MH = 4
VOCAB = 260
B, S = 16, 512
NCORES = 8
RPC = B // NCORES  # rows per core


def _bf16(a):
    import ml_dtypes
    return np.asarray(a, dtype=np.float32).astype(ml_dtypes.bfloat16)


def _pos_enc(s, d):
    pos = np.arange(s, dtype=np.float32)[:, None]
    div = np.exp(np.arange(0, d, 2, dtype=np.float32) * (-math.log(10000.0) / d))
    pe = np.zeros((s, d), np.float32)
    pe[:, 0::2] = np.sin(pos * div)
    pe[:, 1::2] = np.cos(pos * div)
    return pe


def _np(x):
    return np.asarray(x)


def _blocks(w):
    """[K, M] -> [K/P, M/P, P, P] bf16 blocks."""
    K, M = w.shape
    return np.ascontiguousarray(
        _bf16(w).reshape(K // P, P, M // P, P).transpose(0, 2, 1, 3))


def _chunks(v):
    """[d] -> [d/P, P, 1] f32."""
    v = np.asarray(v, np.float32)
    return np.ascontiguousarray(v.reshape(-1, P, 1))


def _dhp(nh, dh):
    """Smallest padded head size (mult of 32) whose per-head row pieces all
    start at partition 0/32/64 and with nh*dhp a multiple of 128."""
    legal = {0, 32, 64}
    cand = ((dh + 31) // 32) * 32
    while True:
        if (nh * cand) % P == 0:
            ok = True
            for h in range(nh):
                start = h * cand
                cnt = cand
                while cnt > 0:
                    r = start % P
                    take = min(P - r, cnt)
                    if r not in legal:
                        ok = False
                    start += take
                    cnt -= take
            if ok:
                return cand
        cand += 32


def _pad_qk(wqkv, bqkv, d, nh):
    """Column-pad each Q/K head to dhp columns (zeros); V unchanged."""
    dh = d // nh
    dhp = _dhp(nh, dh)
    wq, wk, wv = wqkv[:, :d], wqkv[:, d:2 * d], wqkv[:, 2 * d:]
    bq, bk, bv = bqkv[:d], bqkv[d:2 * d], bqkv[2 * d:]

    def padw(w):
        out = np.zeros((d, nh * dhp), np.float32)
        for h in range(nh):
            out[:, h * dhp:h * dhp + dh] = w[:, h * dh:(h + 1) * dh]
        return out

    def padb(b):
        out = np.zeros((nh * dhp,), np.float32)
        for h in range(nh):
            out[h * dhp:h * dhp + dh] = b[h * dh:(h + 1) * dh]
        return out

    wq2, wk2 = padw(wq), padw(wk)
    bq2, bk2 = padb(bq), padb(bk)
    return (np.concatenate([wq2, wk2, wv], axis=1),
            np.concatenate([bq2, bk2, bv]), dhp)


def _pad_wo(wo, d, nh):
    """Row-pad wo to the padded oT layout (zero rows for head padding)."""
    dh = d // nh
    dhp = _dhp(nh, dh)
    out = np.zeros((nh * dhp, d), np.float32)
    for h in range(nh):
        out[h * dhp:h * dhp + dh] = wo[h * dh:(h + 1) * dh]
    return out


def legalize_waits(nc, mybir, bass_rust):
    """This walrus build accepts 1 embedded sem-wait per instruction (2 for
    EventSemaphore); Tile sometimes embeds more. Hoist extras onto inserted
    same-engine EventSemaphore instructions just before the offender."""
    SI = bass_rust.SyncInfo
    n_fixed = 0
    for fn in nc.m.functions:
        for bb in fn.blocks:
            insts = bb.instructions  # live list
            i = 0
            while i < len(insts):
                inst = insts[i]
                si = inst.sync_info
                waits = list(si.on_wait) if si and si.on_wait else []
                cap = 2 if isinstance(inst, mybir.InstEventSemaphore) else 1
                if len(waits) > cap:
                    extra, keep = waits[:-cap], waits[-cap:]
                    si.on_wait = keep
                    inst.sync_info = si
                    for j in range(0, len(extra), 2):
                        es = mybir.InstEventSemaphore(
                            name=f"I-wfix-{nc.next_id()}", ins=[], outs=[])
                        es.engine = inst.engine
                        es.sync_info = SI(on_wait=extra[j:j + 2], on_update=[])
                        insts.insert(i, es)
                        i += 1
                    n_fixed += 1
                i += 1
    return n_fixed


def _prep_host(input_ids, params):
    """Host-side parameter/layout prep. Returns (consts, SP)."""
    ids = _np(input_ids).astype(np.int64)
    p = params
    v = np.arange(VOCAB)
    cont_v = (v >= 128) & (v < 192)
    bnd_v = np.where(cont_v, 0, 1)
    cnt_v = (1 + (v >= 192) + (v >= 224) + (v >= 240)).astype(np.int64)
    emb = np.concatenate([
        _np(p["byte_emb"]),
        _np(p["boundary_emb"])[bnd_v],
        np.broadcast_to(_np(p["ctype_emb"])[0], (VOCAB, 128)),
        _np(p["count_emb"])[cnt_v],
        np.broadcast_to(_np(p["cpos_emb"])[0], (VOCAB, 128)),
    ], axis=1).astype(np.float32)
    table = emb @ _np(p["in_proj"]["w"]) + _np(p["in_proj"]["b"])  # [260, 512]
    posT = np.ascontiguousarray(_pos_enc(S, HD[0]).T)  # [512, 512]

    contm = (ids >= 128) & (ids < 192)
    ng0 = 1 + (~contm[:, 1:]).sum(1)
    SP = int(min(S, ((ng0.max() + 7) // 8) * 8))

    consts = {
        "table": table.astype(np.float32),
        "posT": posT.astype(np.float32),
        "iotab": np.broadcast_to(np.arange(S, dtype=np.float32)[None, :],
                                 (P, S)).copy(),
    }
    scalars = []  # per-layer baked floats (bg2, thr)
    n_layers = len(HD)
    for i, lp in enumerate(_np(params["layers"]) if False else params["layers"]):
        L, M = lp["layer"], lp["merge"]
        d = HD[i]
        pre = f"L{i}_"
        if L["proj"] is not None:
            consts[pre + "proj"] = _blocks(_np(L["proj"]["w"]))
            consts[pre + "projb"] = _chunks(_np(L["proj"]["b"]))
        wq_, bq_, _ = _pad_qk(_np(L["attn"]["wqkv"]), _np(L["attn"]["bqkv"]),
                              d, NH)
        consts[pre + "wqkv"] = _blocks(wq_)
        consts[pre + "bqkv"] = _chunks(bq_)
        consts[pre + "wo"] = _blocks(_pad_wo(_np(L["attn"]["wo"]), d, NH))
        consts[pre + "bo"] = _chunks(_np(L["attn"]["bo"]))
        for nm in ("ln1", "ln2", "norm"):
            consts[pre + nm + "g"] = _chunks(_np(L[nm]["g"]))
            consts[pre + nm + "b"] = _chunks(_np(L[nm]["b"]))
        consts[pre + "ff1"] = _blocks(_np(L["ff1"]["w"]))
        consts[pre + "fb1"] = _chunks(_np(L["ff1"]["b"]))
        consts[pre + "ff2"] = _blocks(_np(L["ff2"]["w"]))
        consts[pre + "fb2"] = _chunks(_np(L["ff2"]["b"]))
        g1 = _np(M["gate1"]["w"])  # [2d, d]
        consts[pre + "g1a"] = _blocks(g1[:d])
        consts[pre + "g1b"] = _blocks(g1[d:])
        consts[pre + "bg1"] = _chunks(_np(M["gate1"]["b"]))
        g2 = _np(M["gate2"]["w"])[:, 0]  # [d]
        consts[pre + "g2"] = np.ascontiguousarray(
            _bf16(g2).reshape(d // P, P, 1))
        mq_, mb_, _ = _pad_qk(_np(M["mattn"]["wqkv"]),
                              _np(M["mattn"]["bqkv"]), d, MH)
        consts[pre + "mqkv"] = _blocks(mq_)
        consts[pre + "mbqkv"] = _chunks(mb_)
        consts[pre + "mwo"] = _blocks(_pad_wo(_np(M["mattn"]["wo"]), d, MH))
        consts[pre + "mbo01"] = _chunks(0.1 * _np(M["mattn"]["bo"]))
        scalars.append({
            "bg2": float(_np(M["gate2"]["b"])[0]),
            "thr": 0.7 + i / n_layers * 0.2,
        })
    wpred = np.concatenate(
        [_np(params["bound_pred"]["w"]), _np(params["ctype_pred"]["w"])], axis=1)
    bpred = np.concatenate(
        [_np(params["bound_pred"]["b"]), _np(params["ctype_pred"]["b"])])
    KT = wpred.shape[0] // P
    consts["wpred"] = np.ascontiguousarray(
        _bf16(wpred).reshape(KT, P, 18))
    consts["bpred"] = np.ascontiguousarray(bpred.astype(np.float32).reshape(18, 1))
    return consts, scalars, SP, ids


def _build(SP, scalars, consts_shapes, n_layers=5):
    import concourse.bass as bass
    import concourse.mybir as mybir
    import concourse.tile as tile
    from concourse.masks import make_identity

    F32 = mybir.dt.float32
    BF16 = mybir.dt.bfloat16
    I32 = mybir.dt.int32
    AF = mybir.ActivationFunctionType
    ALU = mybir.AluOpType

    nc = bass.Bass("TRN2")

    def reg_const(value, dtype=F32):
        key = (dtype, float(value))
        if key in nc.const_aps.aps:
            return
        t = nc.alloc_sbuf_tensor(f"const-{dtype.name}-{value}", [128, 1], dtype)
        nc.gpsimd.memset(t.ap(), float(value))
        nc.const_aps.aps[key] = t.ap()

    reg_const(1e-5)
    for sc in scalars:
        reg_const(sc["bg2"])
    nc.all_engine_barrier()

    # ---- DRAM I/O ----
    dram = {}
    for name, arr in consts_shapes.items():
        dt = {np.dtype(np.float32): F32, np.dtype(np.int32): I32}.get(
            np.dtype(arr.dtype), BF16)
        dram[name] = nc.dram_tensor(name, list(arr.shape), dt,
                                    kind="ExternalInput")
    ids_d = nc.dram_tensor("ids", [RPC, P, S // P], I32, kind="ExternalInput")
    idsf_d = nc.dram_tensor("idsf", [RPC, S], F32, kind="ExternalInput")
    dF = HD[n_layers - 1]
    out_x = nc.dram_tensor("out_x", [RPC, S, dF], F32, kind="ExternalOutput")
    out_bl = nc.dram_tensor("out_bl", [RPC, S, 4], F32, kind="ExternalOutput")
    out_cl = nc.dram_tensor("out_cl", [RPC, S, 14], F32, kind="ExternalOutput")
    out_v = nc.dram_tensor("out_v", [RPC, S], F32, kind="ExternalOutput")

    DTF = dF // P

    with tile.TileContext(nc) as tc:
        ctx_pools = []
        sb = tc.alloc_tile_pool(name="sb", bufs=1)           # persistent
        scr = tc.alloc_tile_pool(name="scr", bufs=3)         # scratch tiles
        wp = tc.alloc_tile_pool(name="wp", bufs=16)          # weight blocks
        rows = tc.alloc_tile_pool(name="rows", bufs=8)       # [1,S] rows
        ps_mm = tc.alloc_tile_pool(name="ps_mm", bufs=2, space="PSUM")
        ps_sc = tc.alloc_tile_pool(name="ps_sc", bufs=2, space="PSUM")
        ps_aux = tc.alloc_tile_pool(name="ps_aux", bufs=2, space="PSUM")
        ps_bc = tc.alloc_tile_pool(name="ps_bc", bufs=1, space="PSUM")

        # ---- constants ----
        idn_f = sb.tile([P, P], F32, name="idn_f")
        make_identity(nc, idn_f[:])
        idn_b = sb.tile([P, P], BF16, name="idn_b")
        make_identity(nc, idn_b[:])
        ones_kb = sb.tile([P, 1], BF16, name="ones_kb")
        nc.vector.memset(ones_kb[:], 1.0)
        ones_row_b = sb.tile([1, P], BF16, name="ones_row_b")
        nc.vector.memset(ones_row_b[:], 1.0)
        one_f = sb.tile([1, 1], F32, name="one_f")
        nc.vector.memset(one_f[:], 1.0)
        zeros_row = sb.tile([1, S], F32, name="zeros_row")
        nc.vector.memset(zeros_row[:], 0.0)
        iotab = sb.tile([P, S], F32, name="iotab")
        nc.sync.dma_start(out=iotab[:], in_=dram["iotab"][:])

        # ---- persistent per-row state ----
        xT = [[sb.tile([P, S], F32, name=f"xT_{r}_{t}") for t in range(DTF)]
              for r in range(RPC)]
        xB = [[sb.tile([P, S], BF16, name=f"xB_{r}_{t}") for t in range(DTF)]
              for r in range(RPC)]
        vrow = [sb.tile([1, S], F32, name=f"vrow_{r}") for r in range(RPC)]
        vcol = [[sb.tile([P, 1], F32, name=f"vcol_{r}_{k}") for k in range(S // P)]
                for r in range(RPC)]

        def vec_col(name, ct):
            """Load [CH,P,1] f32 dram vector chunk ct -> [P,1] sbuf."""
            t = scr.tile([P, 1], F32, name="bias", tag="bias", bufs=6)
            nc.sync.dma_start(out=t[:], in_=dram[name][ct])
            return t

        def row_to_cols(row, n_k, out_dtype=F32):
            """[1, n_k*P] row -> list of [P,1] columns (via K=1 matmuls)."""
            outs = []
            for k in range(n_k):
                ps = ps_aux.tile([P, 1], F32, name="r2c", tag="aux")
                nc.tensor.matmul(ps[:], lhsT=row[0:1, k * P:(k + 1) * P],
                                 rhs=one_f[:], start=True, stop=True)
                o = scr.tile([P, 1], out_dtype, name="col", tag="col", bufs=8)
                nc.vector.tensor_copy(out=o[:], in_=ps[:])
                outs.append(o)
            return outs

        def bcast_row(row_bf, width, tag="bc"):
            """[1,width] bf16 -> [P,width] f32 (SBUF) via outer product."""
            ps = ps_bc.tile([P, width], F32, name="bcast", tag=tag)
            nc.tensor.matmul(ps[:], lhsT=ones_row_b[:], rhs=row_bf[0:1, :width],
                             start=True, stop=True)
            sbt = scr.tile([P, width], F32, name="bcs", tag=f"bcs_{tag}",
                           bufs=2)
            nc.scalar.copy(out=sbt[:], in_=ps[:])
            return sbt

        def head_pieces(start, cnt):
            """Split global row range [start, start+cnt) at 128 boundaries.
            Yields (tile_idx, row_off, n)."""
            pieces = []
            while cnt > 0:
                t = start // P
                r = start % P
                take = min(P - r, cnt)
                pieces.append((t, r, take))
                start += take
                cnt -= take
            return pieces

        def head_rows(tiles, start, cnt, width):
            return [tiles[t][r:r + n, :width]
                    for (t, r, n) in head_pieces(start, cnt)]

        def mm_blocks(wname, KT, mt, rhs_tiles, Sw, ps, start=True, stop=True):
            """psum += W[:,mt-block].T @ rhs  (accumulate over KT k-blocks)."""
            for kt in range(KT):
                blk = wp.tile([P, P], BF16, name="wblk", tag="wblk")
                nc.sync.dma_start(out=blk[:], in_=dram[wname][kt, mt])
                nc.tensor.matmul(ps[:, :Sw], lhsT=blk[:],
                                 rhs=rhs_tiles[kt][:, :Sw],
                                 start=(start and kt == 0),
                                 stop=(stop and kt == KT - 1))

        def linear(wname, bname, KT, MT, rhs_tiles, Sw, evac, extra=None):
            """Generic y = W.T @ rhs + b; evac(mt, psum, bias_col)."""
            for mt in range(MT):
                ps = ps_mm.tile([P, Sw], F32, name="lin", tag="mm")
                mm_blocks(wname, KT, mt, rhs_tiles, Sw, ps)
                if extra is not None:
                    extra(mt, ps)
                bcol = vec_col(bname, mt) if bname else None
                evac(mt, ps, bcol)

        def layernorm(xin, xin_bf, gname, bname, d, Sw, out_tiles):
            """Transposed LN: stats via ones-matmul over bf16 copy."""
            DT = d // P
            ps_sum = ps_aux.tile([1, Sw], F32, name="lnsum", tag="aux")
            for t in range(DT):
                nc.tensor.matmul(ps_sum[:], lhsT=ones_kb[:],
                                 rhs=xin_bf[t][:, :Sw],
                                 start=(t == 0), stop=(t == DT - 1))
            ps_sq = ps_aux.tile([1, Sw], F32, name="lnsq", tag="aux")
            for t in range(DT):
                sq = scr.tile([P, Sw], BF16, name="sq", tag="sq", bufs=3)
                nc.scalar.activation(out=sq[:], in_=xin_bf[t][:, :Sw],
                                     func=AF.Square)
                nc.tensor.matmul(ps_sq[:], lhsT=ones_kb[:], rhs=sq[:],
                                 start=(t == 0), stop=(t == DT - 1))
            mean = rows.tile([1, Sw], F32, name="mean", tag="rows")
            nc.scalar.mul(out=mean[:], in_=ps_sum[:], mul=1.0 / d)
            msq = rows.tile([1, Sw], F32, name="msq", tag="rows")
            nc.scalar.mul(out=msq[:], in_=ps_sq[:], mul=1.0 / d)
            var = rows.tile([1, Sw], F32, name="var", tag="rows")
            m2 = rows.tile([1, Sw], F32, name="m2", tag="rows")
            nc.vector.tensor_tensor(out=m2[:], in0=mean[:], in1=mean[:],
                                    op=ALU.mult)
            nc.vector.tensor_tensor(out=var[:], in0=msq[:], in1=m2[:],
                                    op=ALU.subtract)
            std = rows.tile([1, Sw], F32, name="std", tag="rows")
            nc.scalar.activation(out=std[:], in_=var[:], func=AF.Sqrt,
                                 bias=1e-5)
            rstd = rows.tile([1, Sw], F32, name="rstd", tag="rows")
            nc.vector.reciprocal(out=rstd[:], in_=std[:])
            mrs = rows.tile([1, Sw], F32, name="mrs", tag="rows")
            nc.vector.tensor_tensor(out=mrs[:], in0=mean[:], in1=rstd[:],
                                    op=ALU.mult)
            rstd_b = rows.tile([1, Sw], BF16, name="rstdb", tag="rowsb")
            nc.scalar.copy(out=rstd_b[:], in_=rstd[:])
            mrs_b = rows.tile([1, Sw], BF16, name="mrsb", tag="rowsb")
            nc.scalar.copy(out=mrs_b[:], in_=mrs[:])
            rbc = bcast_row(rstd_b, Sw)
            mbc = bcast_row(mrs_b, Sw, tag="bc2")
            for t in range(DT):
                gcol = vec_col(gname, t)
                bcol = vec_col(bname, t)
                t1 = scr.tile([P, Sw], F32, name="ln_t1", tag="ln_t1", bufs=2)
                nc.vector.tensor_tensor(out=t1[:], in0=xin[t][:, :Sw],
                                        in1=rbc[:], op=ALU.mult)
                t2 = scr.tile([P, Sw], F32, name="ln_t2", tag="ln_t2", bufs=2)
                nc.vector.tensor_tensor(out=t2[:], in0=t1[:], in1=mbc[:],
                                        op=ALU.subtract)
                nc.vector.tensor_scalar(out=out_tiles[t][:, :Sw], in0=t2[:],
                                        scalar1=gcol[:], scalar2=bcol[:],
                                        op0=ALU.mult, op1=ALU.add)

        def attention(r, i, d, nh, qkv_w, qkv_b, Sw, masked):
            """Q/K head-padded attention. Returns oT tiles bf16
            [(nh*dhp)/P][P,S] in the PADDED layout (softmax-normalized);
            wo weights are row-padded to match."""
            DT = d // P
            dh = d // nh
            dhp = _dhp(nh, dh)
            QT = (nh * dhp) // P       # padded Q (and K) tile count
            n_k = (Sw + P - 1) // P
            qkvT = [scr.tile([P, S], BF16, name="qkvT", tag=f"qkvT{t}", bufs=1)
                    for t in range(2 * QT + DT)]
            for mt in range(2 * QT + DT):
                ps = ps_mm.tile([P, Sw], F32, name="qkv", tag="mm")
                mm_blocks(qkv_w, DT, mt, xln_bf, Sw, ps)
                bcol = vec_col(qkv_b, mt)
                nc.scalar.activation(out=qkvT[mt][:, :Sw], in_=ps[:],
                                     func=AF.Identity, bias=bcol[:])
                if Sw < S:
                    nc.vector.memset(qkvT[mt][:, Sw:S], 0.0)
            # V natural: transpose VT blocks
            vnat = [scr.tile([P, d], BF16, name="vnat", tag=f"vnat{k}", bufs=1)
                    for k in range(S // P)]
            for kt in range(n_k):
                for t in range(DT):
                    pst = ps_sc.tile([P, P], BF16, name="vtr", tag="sc")
                    nc.tensor.transpose(
                        out=pst[:],
                        in_=qkvT[2 * QT + t][:, kt * P:(kt + 1) * P],
                        identity=idn_b[:])
                    nc.scalar.copy(out=vnat[kt][:, t * P:(t + 1) * P],
                                   in_=pst[:])
            oT = [scr.tile([P, S], BF16, name="oT", tag=f"oT{t}", bufs=1)
                  for t in range(QT)]
            for t in range(QT):
                nc.vector.memset(oT[t][:], 0.0)  # padded rows must be finite
            for h in range(nh):
                kpieces = head_pieces(h * dhp, dhp)
                e_t = []
                for kt in range(n_k):
                    ps = ps_sc.tile([P, Sw], F32, name="sc", tag="sc")
                    for pi, (t, r0, n) in enumerate(kpieces):
                        nc.tensor.matmul(
                            ps[:],
                            lhsT=qkvT[QT + t][r0:r0 + n, kt * P:(kt + 1) * P],
                            rhs=qkvT[t][r0:r0 + n, :Sw], start=(pi == 0),
                            stop=(pi == len(kpieces) - 1))
                    e = scr.tile([P, Sw], BF16, name="e", tag="e", bufs=4)
                    nc.scalar.activation(out=e[:], in_=ps[:], func=AF.Exp,
                                         scale=1.0 / math.sqrt(dh))
                    if masked:
                        nc.vector.tensor_scalar(
                            out=e[:], in0=e[:], scalar1=vcol[r][kt][:],
                            scalar2=None, op0=ALU.mult)
                    e_t.append(e)
                ps_den = ps_aux.tile([1, Sw], F32, name="den", tag="aux")
                for kt in range(n_k):
                    nc.tensor.matmul(ps_den[:], lhsT=ones_kb[:], rhs=e_t[kt][:],
                                     start=(kt == 0), stop=(kt == n_k - 1))
                rrow = rows.tile([1, Sw], F32, name="rrow", tag="rows")
                nc.vector.reciprocal(out=rrow[:], in_=ps_den[:])
                rrow_b = rows.tile([1, Sw], BF16, name="rrowb", tag="rowsb")
                nc.scalar.copy(out=rrow_b[:], in_=rrow[:])
                rbc = bcast_row(rrow_b, Sw)
                # oT head = (V.T @ e) * recip. Output goes at h*dhp in the
                # padded layout; V columns are at h*dh (unpadded). Chunk so
                # output pieces stay within tiles at legal bases.
                for (t, r0, n_r) in head_pieces(h * dhp, dh):
                    if n_r <= 0:
                        continue
                    coff = (t * P + r0) - h * dhp  # offset within the head
                    if coff >= dh:
                        continue
                    n_r = min(n_r, dh - coff)
                    ps_av = ps_mm.tile([n_r, Sw], F32, name="av", tag="mm")
                    for kt in range(n_k):
                        nc.tensor.matmul(
                            ps_av[:],
                            lhsT=vnat[kt][:, h * dh + coff:h * dh + coff + n_r],
                            rhs=e_t[kt][:], start=(kt == 0),
                            stop=(kt == n_k - 1))
                    nc.vector.tensor_tensor(out=oT[t][r0:r0 + n_r, :Sw],
                                            in0=ps_av[:],
                                            in1=rbc[0:n_r, :],
                                            op=ALU.mult)
            return oT, QT

        # =========== per-row compute ===========
        def embed(r, idx_t):
            """Gather fused table rows, transpose, add pos enc."""
            for jt in range(S // P):
                xg = scr.tile([P, HD[0]], F32, name="xg", tag="xg", bufs=2)
                nc.gpsimd.indirect_dma_start(
                    out=xg[:], out_offset=None, in_=dram["table"][:],
                    in_offset=bass.IndirectOffsetOnAxis(
                        ap=idx_t[:, jt:jt + 1], axis=0))
                for dt in range(HD[0] // P):
                    pst = ps_sc.tile([P, P], F32, name="xtr", tag="sc")
                    nc.tensor.transpose(out=pst[:], in_=xg[:, dt * P:(dt + 1) * P],
                                        identity=idn_f[:])
                    posT_t = scr.tile([P, P], F32, name="pos", tag="pos", bufs=4)
                    nc.sync.dma_start(
                        out=posT_t[:],
                        in_=dram["posT"][dt * P:(dt + 1) * P, jt * P:(jt + 1) * P])
                    nc.vector.tensor_tensor(
                        out=xT[r][dt][:, jt * P:(jt + 1) * P], in0=pst[:],
                        in1=posT_t[:], op=ALU.add)
            for dt in range(HD[0] // P):
                nc.scalar.copy(out=xB[r][dt][:], in_=xT[r][dt][:])

        def enc_layer(r, i, d, Sw):
            DT = d // P
            global xln_bf
            # --- ln1 ---
            xln = [scr.tile([P, S], BF16, name="xln", tag=f"xln{t}", bufs=1)
                   for t in range(DT)]
            layernorm(xT[r], xB[r], f"L{i}_ln1g", f"L{i}_ln1b", d, Sw, xln)
            xln_bf = xln
            oT, QTe = attention(r, i, d, NH, f"L{i}_wqkv", f"L{i}_bqkv", Sw,
                                masked=(i > 0))
            # wo + residual -> h
            hT = [scr.tile([P, S], F32, name="hT", tag=f"hT{t}", bufs=1)
                  for t in range(DT)]
            hB = [scr.tile([P, S], BF16, name="hB", tag=f"hB{t}", bufs=1)
                  for t in range(DT)]

            def evac_wo(mt, ps, bcol):
                nc.vector.scalar_tensor_tensor(
                    out=hT[mt][:, :Sw], in0=ps[:], scalar=bcol[:],
                    in1=xT[r][mt][:, :Sw], op0=ALU.add, op1=ALU.add)
                nc.scalar.copy(out=hB[mt][:, :Sw], in_=hT[mt][:, :Sw])

            linear(f"L{i}_wo", f"L{i}_bo", QTe, DT, oT, Sw, evac_wo)
            # --- ln2 ---
            y2 = [scr.tile([P, S], BF16, name="y2", tag=f"xln{t}", bufs=1)
                  for t in range(DT)]
            layernorm(hT, hB, f"L{i}_ln2g", f"L{i}_ln2b", d, Sw, y2)
            xln_bf = y2
            # --- ff --- (gelu tiles alias dead qkvT/oT slots to save SBUF)
            def _gtag(j):
                return f"qkvT{j}" if j < 3 * DT else f"oT{j - 3 * DT}"
            gbf = [scr.tile([P, S], BF16, name="gbf", tag=_gtag(t), bufs=1)
                   for t in range(4 * DT)]

            def evac_gelu(mt, ps, bcol):
                nc.scalar.activation(out=gbf[mt][:, :Sw], in_=ps[:],
                                     func=AF.Gelu, bias=bcol[:])

            linear(f"L{i}_ff1", f"L{i}_fb1", DT, 4 * DT, xln_bf, Sw, evac_gelu)

            def evac_ff2(mt, ps, bcol):
                nc.vector.scalar_tensor_tensor(
                    out=hT[mt][:, :Sw], in0=ps[:], scalar=bcol[:],
                    in1=hT[mt][:, :Sw], op0=ALU.add, op1=ALU.add)
                nc.scalar.copy(out=hB[mt][:, :Sw], in_=hT[mt][:, :Sw])

            linear(f"L{i}_ff2", f"L{i}_fb2", 4 * DT, DT, gbf, Sw, evac_ff2)
            # --- final norm -> xB (bf16 only) ---
            layernorm(hT, hB, f"L{i}_normg", f"L{i}_normb", d, Sw,
                      [xB[r][t] for t in range(DT)])

        def merge_layer(r, i, d, Sw, idsf_row):
            """Gates, gid, merge attention mean, segment mean, combine."""
            DT = d // P
            n_k = (Sw + P - 1) // P
            global xln_bf
            xln_bf = xB[r]
            # ---- gates (probs unused at layer 0: decision comes from ids) ----
            if i > 0:
                gre = [scr.tile([P, S], BF16, name="gre", tag=f"oT{t}", bufs=1)
                       for t in range(DT)]
                for mt in range(DT):
                    ps = ps_mm.tile([P, Sw], F32, name="g1", tag="mm")
                    mm_blocks(f"L{i}_g1a", DT, mt,
                              [xB[r][t][:, 0:S] for t in range(DT)], Sw - 1, ps,
                              stop=False)
                    mm_blocks(f"L{i}_g1b", DT, mt,
                              [xB[r][t][:, 1:S] for t in range(DT)], Sw - 1, ps,
                              start=False)
                    bcol = vec_col(f"L{i}_bg1", mt)
                    nc.scalar.activation(out=gre[mt][:, :Sw - 1],
                                         in_=ps[:, :Sw - 1],
                                         func=AF.Relu, bias=bcol[:])
                ps_pr = ps_aux.tile([1, Sw - 1], F32, name="pr", tag="aux")
                for t in range(DT):
                    g2c = scr.tile([P, 1], BF16, name="g2c", tag="col2", bufs=8)
                    nc.sync.dma_start(out=g2c[:], in_=dram[f"L{i}_g2"][t])
                    nc.tensor.matmul(ps_pr[:], lhsT=g2c[:],
                                     rhs=gre[t][:, :Sw - 1],
                                     start=(t == 0), stop=(t == DT - 1))
            dec = rows.tile([1, S], F32, name="dec", tag="rows")
            if i == 0:
                # dec from ids: continuation bytes merge
                c1 = rows.tile([1, S], F32, name="c1", tag="rows")
                nc.vector.tensor_scalar(out=c1[:], in0=idsf_row[:],
                                        scalar1=127.5, scalar2=None,
                                        op0=ALU.is_gt)
                c2 = rows.tile([1, S], F32, name="c2", tag="rows")
                nc.vector.tensor_scalar(out=c2[:], in0=idsf_row[:],
                                        scalar1=191.5, scalar2=None,
                                        op0=ALU.is_lt)
                co = rows.tile([1, S], F32, name="co", tag="rows")
                nc.vector.tensor_tensor(out=co[:], in0=c1[:], in1=c2[:],
                                        op=ALU.mult)
                nc.vector.tensor_copy(out=dec[0:1, 0:Sw - 1],
                                      in_=co[0:1, 1:Sw])
            else:
                pr = rows.tile([1, Sw - 1], F32, name="prs", tag="rows")
                nc.scalar.activation(out=pr[:], in_=ps_pr[:], func=AF.Sigmoid,
                                     bias=scalars[i]["bg2"])
                nc.vector.tensor_scalar(out=dec[0:1, 0:Sw - 1], in0=pr[:],
                                        scalar1=scalars[i]["thr"],
                                        scalar2=None, op0=ALU.is_gt)
            # mask by valid[1:]
            decm = rows.tile([1, S], F32, name="decm", tag="rows")
            nc.vector.tensor_tensor(out=decm[0:1, 0:Sw - 1],
                                    in0=dec[0:1, 0:Sw - 1],
                                    in1=vrow[r][0:1, 1:Sw], op=ALU.mult)
            # m = 1 - dec (gid increments); mfull[0] = 0
            mfull = rows.tile([1, S], F32, name="mfull", tag="rows")
            nc.vector.memset(mfull[:], 0.0)
            nc.vector.tensor_scalar(out=mfull[0:1, 1:Sw],
                                    in0=decm[0:1, 0:Sw - 1], scalar1=-1.0,
                                    scalar2=1.0, op0=ALU.mult, op1=ALU.add)
            # n_groups = 1 + sum(m * valid[1:])
            mng = rows.tile([1, S], F32, name="mng", tag="rows")
            nc.vector.tensor_tensor(out=mng[0:1, 0:Sw - 1],
                                    in0=mfull[0:1, 1:Sw],
                                    in1=vrow[r][0:1, 1:Sw], op=ALU.mult)
            ng = rows.tile([1, 1], F32, name="ng", tag="ng", bufs=2)
            nc.vector.tensor_reduce(out=ng[:], in_=mng[0:1, 0:Sw - 1],
                                    axis=mybir.AxisListType.X, op=ALU.add)
            nc.vector.tensor_scalar(out=ng[:], in0=ng[:], scalar1=1.0,
                                    scalar2=None, op0=ALU.add)
            gid = rows.tile([1, S], F32, name="gid", tag="rows")
            nc.vector.memset(gid[:], 60000.0)  # tail never matches any group
            nc.vector.tensor_tensor_scan(
                out=gid[0:1, 0:Sw], data0=mfull[0:1, 0:Sw],
                data1=zeros_row[0:1, 0:Sw], initial=0.0,
                op0=ALU.add, op1=ALU.add)
            gcols = row_to_cols(gid, n_k)
            # MT[j, g] = (gid[j] == g), bf16
            SPW = SP
            MTt = [scr.tile([P, SPW], BF16, name="MT", tag=f"MT{k}", bufs=1)
                   for k in range(n_k)]
            for k in range(n_k):
                nc.vector.tensor_scalar(out=MTt[k][:], in0=iotab[:, :SPW],
                                        scalar1=gcols[k][:], scalar2=None,
                                        op0=ALU.is_equal)
            # counts + recip + new valid
            ps_cnt = ps_aux.tile([1, SPW], F32, name="cnt", tag="aux")
            for k in range(n_k):
                nc.tensor.matmul(ps_cnt[:], lhsT=ones_kb[:], rhs=MTt[k][:],
                                 start=(k == 0), stop=(k == n_k - 1))
            cclip = rows.tile([1, SPW], F32, name="cclip", tag="rows")
            nc.vector.tensor_scalar(out=cclip[:], in0=ps_cnt[:], scalar1=1.0,
                                    scalar2=None, op0=ALU.max)
            crec = rows.tile([1, SPW], F32, name="crec", tag="rows")
            nc.vector.reciprocal(out=crec[:], in_=cclip[:])
            nv = rows.tile([1, S], F32, name="nv", tag="nv", bufs=2)
            nc.vector.memset(nv[:], 0.0)
            nc.vector.tensor_scalar(out=nv[0:1, 0:SPW], in0=iotab[0:1, 0:SPW],
                                    scalar1=ng[:], scalar2=None, op0=ALU.is_lt)
            rv = rows.tile([1, SPW], F32, name="rv", tag="rows")
            nc.vector.tensor_tensor(out=rv[:], in0=crec[:],
                                    in1=nv[0:1, 0:SPW], op=ALU.mult)
            rvb = rows.tile([1, SPW], BF16, name="rvb", tag="rowsb")
            nc.scalar.copy(out=rvb[:], in_=rv[:])
            nvb = rows.tile([1, SPW], BF16, name="nvb", tag="rowsb")
            nc.scalar.copy(out=nvb[:], in_=nv[0:1, 0:SPW])
            # ---- merge attention (mean only) ----
            oTm, QTm = attention(r, i, d, MH, f"L{i}_mqkv", f"L{i}_mbqkv", Sw,
                                 masked=(i > 0))
            # masked mean over positions: am[dcol] = sum_j oT * vmask / Nv
            nv_cur = rows.tile([1, 1], F32, name="nv_cur", tag="ng", bufs=2)
            nc.vector.tensor_reduce(out=nv_cur[:], in_=vrow[r][0:1, 0:Sw],
                                    axis=mybir.AxisListType.X, op=ALU.add)
            nvrec = rows.tile([1, 1], F32, name="nvrec", tag="ng", bufs=2)
            nc.vector.reciprocal(out=nvrec[:], in_=nv_cur[:])
            nvrec_b = rows.tile([1, 1], BF16, name="nvrec_b", tag="ngb",
                                bufs=2)
            nc.scalar.copy(out=nvrec_b[:], in_=nvrec[:])
            ps_nb = ps_aux.tile([P, 1], F32, name="ps_nb", tag="aux")
            nc.tensor.matmul(ps_nb[:], lhsT=ones_row_b[:], rhs=nvrec_b[:],
                             start=True, stop=True)
            nvrec_c = scr.tile([P, 1], F32, name="nvrec_c", tag="col", bufs=8)
            nc.scalar.copy(out=nvrec_c[:], in_=ps_nb[:])
            vrow_b = rows.tile([1, S], BF16, name="vrowb", tag="rowsb")
            nc.scalar.copy(out=vrow_b[0:1, 0:Sw], in_=vrow[r][0:1, 0:Sw])
            vbc = bcast_row(vrow_b, Sw)
            om = []  # [QTm][P,1] bf16: mean of o over valid positions
            for t in range(QTm):
                tmp = scr.tile([P, Sw], F32, name="omtmp", tag="omtmp", bufs=2)
                nc.vector.tensor_tensor(out=tmp[:], in0=oTm[t][:, :Sw],
                                        in1=vbc[:], op=ALU.mult)
                o1 = scr.tile([P, 1], F32, name="om1", tag="col", bufs=8)
                nc.vector.tensor_reduce(out=o1[:], in_=tmp[:],
                                        axis=mybir.AxisListType.X, op=ALU.add)
                ob = scr.tile([P, 1], BF16, name="omb", tag="col2", bufs=8)
                nc.vector.tensor_scalar(out=ob[:], in0=o1[:],
                                        scalar1=nvrec_c[:], scalar2=0.1,
                                        op0=ALU.mult, op1=ALU.mult)
                om.append(ob)
            # am = mwo.T @ om + 0.1*mbo  -> [DT][P,1] f32
            am = []
            for mt in range(DT):
                ps = ps_aux.tile([P, 1], F32, name="am", tag="aux")
                for kt in range(QTm):
                    blk = wp.tile([P, P], BF16, name="wblk", tag="wblk")
                    nc.sync.dma_start(out=blk[:], in_=dram[f"L{i}_mwo"][kt, mt])
                    nc.tensor.matmul(ps[:], lhsT=blk[:], rhs=om[kt][:],
                                     start=(kt == 0), stop=(kt == DT - 1))
                bcol = vec_col(f"L{i}_mbo01", mt)
                a = scr.tile([P, 1], F32, name="amc", tag="col", bufs=8)
                nc.scalar.activation(out=a[:], in_=ps[:], func=AF.Identity,
                                     bias=bcol[:])
                am.append(a)
            # ---- segment mean + combine ----
            xnat = [scr.tile([P, d], BF16, name="xnat", tag=f"vnat{k}", bufs=1)
                    for k in range(n_k)]
            for k in range(n_k):
                for t in range(DT):
                    pst = ps_sc.tile([P, P], BF16, name="xtr2", tag="sc")
                    nc.tensor.transpose(out=pst[:],
                                        in_=xB[r][t][:, k * P:(k + 1) * P],
                                        identity=idn_b[:])
                    nc.scalar.copy(out=xnat[k][:, t * P:(t + 1) * P], in_=pst[:])
            rvbc = bcast_row(rvb, SPW)
            nvbc = bcast_row(nvb, SPW, tag="bc2")
            for dt in range(DT):
                ps = ps_mm.tile([P, SPW], F32, name="seg", tag="mm")
                for k in range(n_k):
                    nc.tensor.matmul(ps[:], lhsT=xnat[k][:, dt * P:(dt + 1) * P],
                                     rhs=MTt[k][:], start=(k == 0),
                                     stop=(k == n_k - 1))
                t1 = scr.tile([P, SPW], F32, name="cmb", tag="cmb", bufs=2)
                nc.vector.tensor_tensor(out=t1[:], in0=ps[:], in1=rvbc[:],
                                        op=ALU.mult)
                nc.vector.scalar_tensor_tensor(
                    out=xT[r][dt][:, 0:SPW], in0=t1[:], scalar=am[dt][:],
                    in1=nvbc[:], op0=ALU.add, op1=ALU.mult)
                nc.scalar.copy(out=xB[r][dt][:, 0:SPW],
                               in_=xT[r][dt][:, 0:SPW])
            # update valid state
            nc.vector.tensor_copy(out=vrow[r][:], in_=nv[:])
            nvcols = row_to_cols(nv, S // P)
            for k in range(S // P):
                nc.vector.tensor_copy(out=vcol[r][k][:], in_=nvcols[k][:])

        def proj_layer(r, i, d_in, d, Sw):
            """x = x @ proj + b (changes width d_in -> d)."""
            newT = [scr.tile([P, S], F32, name="pj", tag=f"hT{t}", bufs=1)
                    for t in range(d // P)]

            def evac(mt, ps, bcol):
                nc.scalar.activation(out=newT[mt][:, :Sw], in_=ps[:],
                                     func=AF.Identity, bias=bcol[:])

            linear(f"L{i}_proj", f"L{i}_projb", d_in // P, d // P,
                   xB[r], Sw, evac)
            for t in range(d // P):
                nc.vector.tensor_copy(out=xT[r][t][:, :Sw],
                                      in_=newT[t][:, :Sw])
                nc.scalar.copy(out=xB[r][t][:, :Sw], in_=newT[t][:, :Sw])

        def outputs(r, Sw):
            """Final x transpose + preds + DMA out."""
            # zero tails so outputs beyond SP are exact zeros
            for t in range(DTF):
                if Sw < S:
                    nc.vector.memset(xT[r][t][:, Sw:S], 0.0)
                    nc.vector.memset(xB[r][t][:, Sw:S], 0.0)
            # preds: [18, S] = wpred.T @ xB + b
            ps_p = ps_aux.tile([18, S], F32, name="pred", tag="aux")
            for t in range(DTF):
                wchunk = scr.tile([P, 18], BF16, name="wpr", tag="col2", bufs=8)
                nc.sync.dma_start(out=wchunk[:], in_=dram["wpred"][t])
                nc.tensor.matmul(ps_p[:], lhsT=wchunk[:], rhs=xB[r][t][:],
                                 start=(t == 0), stop=(t == DTF - 1))
            bpr = scr.tile([18, 1], F32, name="bpr", tag="col", bufs=8)
            nc.sync.dma_start(out=bpr[:], in_=dram["bpred"][:])
            predT = scr.tile([18, S], F32, name="predT", tag="predT", bufs=1)
            nc.scalar.activation(out=predT[:], in_=ps_p[:], func=AF.Identity,
                                 bias=bpr[:])
            for jt in range(S // P):
                # x natural out
                xno = scr.tile([P, dF], F32, name="xno", tag="xno", bufs=2)
                for t in range(DTF):
                    pst = ps_sc.tile([P, P], F32, name="xotr", tag="sc")
                    nc.tensor.transpose(out=pst[:],
                                        in_=xT[r][t][:, jt * P:(jt + 1) * P],
                                        identity=idn_f[:])
                    nc.scalar.copy(out=xno[:, t * P:(t + 1) * P], in_=pst[:])
                nc.sync.dma_start(out=out_x[r, jt * P:(jt + 1) * P, :],
                                  in_=xno[:])
                pst = ps_sc.tile([P, 18], F32, name="ptr", tag="sc")
                nc.tensor.matmul(pst[:], lhsT=predT[:, jt * P:(jt + 1) * P],
                                 rhs=idn_f[0:18, 0:18], start=True, stop=True)
                pno = scr.tile([P, 18], F32, name="pno", tag="pno", bufs=2)
                nc.scalar.copy(out=pno[:], in_=pst[:])
                nc.sync.dma_start(out=out_bl[r, jt * P:(jt + 1) * P, :],
                                  in_=pno[:, 0:4])
                nc.sync.dma_start(out=out_cl[r, jt * P:(jt + 1) * P, :],
                                  in_=pno[:, 4:18])
            nc.sync.dma_start(out=out_v[r:r + 1, :], in_=vrow[r][0:1, :])

        # ================= main program =================
        for r in range(RPC):
            idx_t = scr.tile([P, S // P], I32, name="idx", tag="idx", bufs=2)
            nc.sync.dma_start(out=idx_t[:], in_=ids_d[r])
            idsf_row = sb.tile([1, S], F32, name=f"idsf_{r}")
            nc.sync.dma_start(out=idsf_row[:], in_=idsf_d[r:r + 1, :])
            embed(r, idx_t)
            nc.vector.memset(vrow[r][:], 1.0)
            for k in range(S // P):
                nc.vector.memset(vcol[r][k][:], 1.0)
            Sw = S
            for i in range(n_layers):
                d_in = HD[i - 1] if i > 0 else HD[0]
                d = HD[i]
                if d_in != d:
                    proj_layer(r, i, d_in, d, Sw)
                enc_layer(r, i, d, Sw)
                merge_layer(r, i, d, Sw, idsf_row)
                Sw = SP
            outputs(r, Sw)

        for pl in (ps_bc, ps_aux, ps_sc, ps_mm, rows, wp, scr, sb):
            pl.release()

    return nc


def kernel(input_ids, params):
    import concourse.bass as bass  # noqa
    import concourse.mybir as mybir
    import bass_rust
    from concourse.bass_utils import run_bass_kernel_spmd

    consts, scalars, SP, ids = _prep_host(input_ids, params)
    n_layers = len(HD)
    nc = _build(SP, scalars, consts, n_layers=n_layers)
    legalize_waits(nc, mybir, bass_rust)

    # per-core inputs
    ids32 = ids.astype(np.int32)
    idssb = ids32.reshape(B, S // P, P).transpose(0, 2, 1)  # [B, P, S/P]
    idsf = ids32.astype(np.float32)
    in_maps = []
    for c in range(NCORES):
        m = dict(consts)
        m["ids"] = np.ascontiguousarray(idssb[c * RPC:(c + 1) * RPC])
        m["idsf"] = np.ascontiguousarray(idsf[c * RPC:(c + 1) * RPC])
        in_maps.append(m)
    import os
    import time as _time
    res = run_bass_kernel_spmd(nc, in_maps, list(range(NCORES))).results
    if os.environ.get("BASS_KERNEL_TIME"):
        # second run hits the warm jit/NEFF cache: wall time ~= dispatch +
        # input transfer + execute
        t0 = _time.time()
        res = run_bass_kernel_spmd(nc, in_maps, list(range(NCORES))).results
        print(f"warm rerun wall: {(_time.time() - t0) * 1e9:.0f} ns")

    dF = HD[n_layers - 1]
    x = np.zeros((B, S, dF), np.float32)
    bl = np.zeros((B, S, 4), np.float32)
    cl = np.zeros((B, S, 14), np.float32)
    vv = np.zeros((B, S), np.float32)
    for c in range(NCORES):
        x[c * RPC:(c + 1) * RPC] = res[c]["out_x"]
        bl[c * RPC:(c + 1) * RPC] = res[c]["out_bl"]
        cl[c * RPC:(c + 1) * RPC] = res[c]["out_cl"]
        vv[c * RPC:(c + 1) * RPC] = res[c]["out_v"]
    return x, bl, cl, vv


# revision 31
# speedup vs baseline: 1.1421x; 1.1421x over previous
"""BoundaryAwareEncoder Trainium2 kernel.

Data-parallel over batch: 16 rows -> 8 cores x 2 rows. Activations kept
TRANSPOSED in SBUF (xT [d, S], d on partitions) so every linear uses the
natural weight layout as lhsT. Softmax runs with keys on partitions (scores
magnitude <= ~1.5 so no max-subtraction needed). Segment-mean is a one-hot
matmul built on-device from a cumsum (tensor_tensor_scan) of the merge
decisions. bf16 matmuls with fp32 PSUM accumulation throughout.
"""
import math
import numpy as np

P = 128
HD = [512, 512, 640, 768, 768]
NH = 8
MH = 4
VOCAB = 260
B, S = 16, 512
NCORES = 8
RPC = B // NCORES  # rows per core


def _bf16(a):
    import ml_dtypes
    return np.asarray(a, dtype=np.float32).astype(ml_dtypes.bfloat16)


def _pos_enc(s, d):
    pos = np.arange(s, dtype=np.float32)[:, None]
    div = np.exp(np.arange(0, d, 2, dtype=np.float32) * (-math.log(10000.0) / d))
    pe = np.zeros((s, d), np.float32)
    pe[:, 0::2] = np.sin(pos * div)
    pe[:, 1::2] = np.cos(pos * div)
    return pe


def _np(x):
    return np.asarray(x)


def _blocks(w):
    """[K, M] -> [K/P, M/P, P, P] bf16 blocks."""
    K, M = w.shape
    return np.ascontiguousarray(
        _bf16(w).reshape(K // P, P, M // P, P).transpose(0, 2, 1, 3))


def _chunks(v):
    """[d] -> [d/P, P, 1] f32."""
    v = np.asarray(v, np.float32)
    return np.ascontiguousarray(v.reshape(-1, P, 1))


def _dhp(nh, dh):
    """Smallest padded head size (mult of 32) whose per-head row pieces all
    start at partition 0/32/64 and with nh*dhp a multiple of 128."""
    legal = {0, 32, 64}
    cand = ((dh + 31) // 32) * 32
    while True:
        if (nh * cand) % P == 0:
            ok = True
            for h in range(nh):
                start = h * cand
                cnt = cand
                while cnt > 0:
                    r = start % P
                    take = min(P - r, cnt)
                    if r not in legal:
                        ok = False
                    start += take
                    cnt -= take
            if ok:
                return cand
        cand += 32


def _pad_qk(wqkv, bqkv, d, nh):
    """Column-pad each Q/K head to dhp columns (zeros); V unchanged."""
    dh = d // nh
    dhp = _dhp(nh, dh)
    wq, wk, wv = wqkv[:, :d], wqkv[:, d:2 * d], wqkv[:, 2 * d:]
    bq, bk, bv = bqkv[:d], bqkv[d:2 * d], bqkv[2 * d:]

    def padw(w):
        out = np.zeros((d, nh * dhp), np.float32)
        for h in range(nh):
            out[:, h * dhp:h * dhp + dh] = w[:, h * dh:(h + 1) * dh]
        return out

    def padb(b):
        out = np.zeros((nh * dhp,), np.float32)
        for h in range(nh):
            out[h * dhp:h * dhp + dh] = b[h * dh:(h + 1) * dh]
        return out

    wq2, wk2 = padw(wq), padw(wk)
    bq2, bk2 = padb(bq), padb(bk)
    return (np.concatenate([wq2, wk2, wv], axis=1),
            np.concatenate([bq2, bk2, bv]), dhp)


def _pad_wo(wo, d, nh):
    """Row-pad wo to the padded oT layout (zero rows for head padding)."""
    dh = d // nh
    dhp = _dhp(nh, dh)
    out = np.zeros((nh * dhp, d), np.float32)
    for h in range(nh):
        out[h * dhp:h * dhp + dh] = wo[h * dh:(h + 1) * dh]
    return out


def legalize_waits(nc, mybir, bass_rust):
    """This walrus build accepts 1 embedded sem-wait per instruction (2 for
    EventSemaphore); Tile sometimes embeds more. Hoist extras onto inserted
    same-engine EventSemaphore instructions just before the offender."""
    SI = bass_rust.SyncInfo
    n_fixed = 0
    for fn in nc.m.functions:
        for bb in fn.blocks:
            insts = bb.instructions  # live list
            i = 0
            while i < len(insts):
                inst = insts[i]
                si = inst.sync_info
                waits = list(si.on_wait) if si and si.on_wait else []
                cap = 2 if isinstance(inst, mybir.InstEventSemaphore) else 1
                if len(waits) > cap:
                    extra, keep = waits[:-cap], waits[-cap:]
                    si.on_wait = keep
                    inst.sync_info = si
                    for j in range(0, len(extra), 2):
                        es = mybir.InstEventSemaphore(
                            name=f"I-wfix-{nc.next_id()}", ins=[], outs=[])
                        es.engine = inst.engine
                        es.sync_info = SI(on_wait=extra[j:j + 2], on_update=[])
                        insts.insert(i, es)
                        i += 1
                    n_fixed += 1
                i += 1
    return n_fixed


def _prep_host(input_ids, params):
    """Host-side parameter/layout prep. Returns (consts, SP)."""
    ids = _np(input_ids).astype(np.int64)
    p = params
    v = np.arange(VOCAB)
    cont_v = (v >= 128) & (v < 192)
    bnd_v = np.where(cont_v, 0, 1)
    cnt_v = (1 + (v >= 192) + (v >= 224) + (v >= 240)).astype(np.int64)
    emb = np.concatenate([
        _np(p["byte_emb"]),
        _np(p["boundary_emb"])[bnd_v],
        np.broadcast_to(_np(p["ctype_emb"])[0], (VOCAB, 128)),
        _np(p["count_emb"])[cnt_v],
        np.broadcast_to(_np(p["cpos_emb"])[0], (VOCAB, 128)),
    ], axis=1).astype(np.float32)
    table = emb @ _np(p["in_proj"]["w"]) + _np(p["in_proj"]["b"])  # [260, 512]
    posT = np.ascontiguousarray(_pos_enc(S, HD[0]).T)  # [512, 512]

    contm = (ids >= 128) & (ids < 192)
    ng0 = 1 + (~contm[:, 1:]).sum(1)
    SP = int(min(S, ((ng0.max() + 7) // 8) * 8))

    consts = {
        "table": table.astype(np.float32),
        "posT": posT.astype(np.float32),
        "iotab": np.broadcast_to(np.arange(S, dtype=np.float32)[None, :],
                                 (P, S)).copy(),
    }
    scalars = []  # per-layer baked floats (bg2, thr)
    n_layers = len(HD)
    for i, lp in enumerate(_np(params["layers"]) if False else params["layers"]):
        L, M = lp["layer"], lp["merge"]
        d = HD[i]
        pre = f"L{i}_"
        if L["proj"] is not None:
            consts[pre + "proj"] = _blocks(_np(L["proj"]["w"]))
            consts[pre + "projb"] = _chunks(_np(L["proj"]["b"]))
        wq_, bq_, _ = _pad_qk(_np(L["attn"]["wqkv"]), _np(L["attn"]["bqkv"]),
                              d, NH)
        consts[pre + "wqkv"] = _blocks(wq_)
        consts[pre + "bqkv"] = _chunks(bq_)
        consts[pre + "wo"] = _blocks(_pad_wo(_np(L["attn"]["wo"]), d, NH))
        consts[pre + "bo"] = _chunks(_np(L["attn"]["bo"]))
        for nm in ("ln1", "ln2", "norm"):
            consts[pre + nm + "g"] = _chunks(_np(L[nm]["g"]))
            consts[pre + nm + "b"] = _chunks(_np(L[nm]["b"]))
        consts[pre + "ff1"] = _blocks(_np(L["ff1"]["w"]))
        consts[pre + "fb1"] = _chunks(_np(L["ff1"]["b"]))
        consts[pre + "ff2"] = _blocks(_np(L["ff2"]["w"]))
        consts[pre + "fb2"] = _chunks(_np(L["ff2"]["b"]))
        g1 = _np(M["gate1"]["w"])  # [2d, d]
        consts[pre + "g1a"] = _blocks(g1[:d])
        consts[pre + "g1b"] = _blocks(g1[d:])
        consts[pre + "bg1"] = _chunks(_np(M["gate1"]["b"]))
        g2 = _np(M["gate2"]["w"])[:, 0]  # [d]
        consts[pre + "g2"] = np.ascontiguousarray(
            _bf16(g2).reshape(d // P, P, 1))
        mq_, mb_, _ = _pad_qk(_np(M["mattn"]["wqkv"]),
                              _np(M["mattn"]["bqkv"]), d, MH)
        consts[pre + "mqkv"] = _blocks(mq_)
        consts[pre + "mbqkv"] = _chunks(mb_)
        consts[pre + "mwo"] = _blocks(_pad_wo(_np(M["mattn"]["wo"]), d, MH))
        consts[pre + "mbo01"] = _chunks(0.1 * _np(M["mattn"]["bo"]))
        scalars.append({
            "bg2": float(_np(M["gate2"]["b"])[0]),
            "thr": 0.7 + i / n_layers * 0.2,
        })
    wpred = np.concatenate(
        [_np(params["bound_pred"]["w"]), _np(params["ctype_pred"]["w"])], axis=1)
    bpred = np.concatenate(
        [_np(params["bound_pred"]["b"]), _np(params["ctype_pred"]["b"])])
    KT = wpred.shape[0] // P
    consts["wpred"] = np.ascontiguousarray(
        _bf16(wpred).reshape(KT, P, 18))
    consts["bpred"] = np.ascontiguousarray(bpred.astype(np.float32).reshape(18, 1))
    return consts, scalars, SP, ids


def _build(SP, scalars, consts_shapes, n_layers=5):
    import concourse.bass as bass
    import concourse.mybir as mybir
    import concourse.tile as tile
    from concourse.masks import make_identity

    F32 = mybir.dt.float32
    BF16 = mybir.dt.bfloat16
    I32 = mybir.dt.int32
    AF = mybir.ActivationFunctionType
    ALU = mybir.AluOpType

    nc = bass.Bass("TRN2")

    def reg_const(value, dtype=F32):
        key = (dtype, float(value))
        if key in nc.const_aps.aps:
            return
        t = nc.alloc_sbuf_tensor(f"const-{dtype.name}-{value}", [128, 1], dtype)
        nc.gpsimd.memset(t.ap(), float(value))
        nc.const_aps.aps[key] = t.ap()

    reg_const(1e-5)
    for sc in scalars:
        reg_const(sc["bg2"])
    nc.all_engine_barrier()

    # ---- DRAM I/O ----
    dram = {}
    for name, arr in consts_shapes.items():
        dt = {np.dtype(np.float32): F32, np.dtype(np.int32): I32}.get(
            np.dtype(arr.dtype), BF16)
        dram[name] = nc.dram_tensor(name, list(arr.shape), dt,
                                    kind="ExternalInput")
    ids_d = nc.dram_tensor("ids", [RPC, P, S // P], I32, kind="ExternalInput")
    idsf_d = nc.dram_tensor("idsf", [RPC, S], F32, kind="ExternalInput")
    dF = HD[n_layers - 1]
    out_x = nc.dram_tensor("out_x", [RPC, S, dF], F32, kind="ExternalOutput")
    out_bl = nc.dram_tensor("out_bl", [RPC, S, 4], F32, kind="ExternalOutput")
    out_cl = nc.dram_tensor("out_cl", [RPC, S, 14], F32, kind="ExternalOutput")
    out_v = nc.dram_tensor("out_v", [RPC, S], F32, kind="ExternalOutput")

    DTF = dF // P

    with tile.TileContext(nc) as tc:
        ctx_pools = []
        sb = tc.alloc_tile_pool(name="sb", bufs=1)           # persistent
        scr = tc.alloc_tile_pool(name="scr", bufs=3)         # scratch tiles
        wp = tc.alloc_tile_pool(name="wp", bufs=64)          # weight blocks
        rows = tc.alloc_tile_pool(name="rows", bufs=8)       # [1,S] rows
        ps_mm = tc.alloc_tile_pool(name="ps_mm", bufs=2, space="PSUM")
        ps_sc = tc.alloc_tile_pool(name="ps_sc", bufs=2, space="PSUM")
        ps_aux = tc.alloc_tile_pool(name="ps_aux", bufs=2, space="PSUM")
        ps_bc = tc.alloc_tile_pool(name="ps_bc", bufs=1, space="PSUM")

        # ---- constants ----
        idn_f = sb.tile([P, P], F32, name="idn_f")
        make_identity(nc, idn_f[:])
        idn_b = sb.tile([P, P], BF16, name="idn_b")
        make_identity(nc, idn_b[:])
        ones_kb = sb.tile([P, 1], BF16, name="ones_kb")
        nc.vector.memset(ones_kb[:], 1.0)
        ones_row_b = sb.tile([1, P], BF16, name="ones_row_b")
        nc.vector.memset(ones_row_b[:], 1.0)
        one_f = sb.tile([1, 1], F32, name="one_f")
        nc.vector.memset(one_f[:], 1.0)
        zeros_row = sb.tile([1, S], F32, name="zeros_row")
        nc.vector.memset(zeros_row[:], 0.0)
        iotab = sb.tile([P, S], F32, name="iotab")
        nc.sync.dma_start(out=iotab[:], in_=dram["iotab"][:])

        # ---- persistent per-row state ----
        xT = [[sb.tile([P, S], F32, name=f"xT_{r}_{t}") for t in range(DTF)]
              for r in range(RPC)]
        xB = [[sb.tile([P, S], BF16, name=f"xB_{r}_{t}") for t in range(DTF)]
              for r in range(RPC)]
        vrow = [sb.tile([1, S], F32, name=f"vrow_{r}") for r in range(RPC)]
        vcol = [[sb.tile([P, 1], F32, name=f"vcol_{r}_{k}") for k in range(S // P)]
                for r in range(RPC)]

        def vec_col(name, ct):
            """Load [CH,P,1] f32 dram vector chunk ct -> [P,1] sbuf."""
            t = scr.tile([P, 1], F32, name="bias", tag="bias", bufs=12)
            nc.sync.dma_start(out=t[:], in_=dram[name][ct])
            return t

        def row_to_cols(row, n_k, out_dtype=F32):
            """[1, n_k*P] row -> list of [P,1] columns (via K=1 matmuls)."""
            outs = []
            for k in range(n_k):
                ps = ps_aux.tile([P, 1], F32, name="r2c", tag="aux")
                nc.tensor.matmul(ps[:], lhsT=row[0:1, k * P:(k + 1) * P],
                                 rhs=one_f[:], start=True, stop=True)
                o = scr.tile([P, 1], out_dtype, name="col", tag="col", bufs=8)
                nc.vector.tensor_copy(out=o[:], in_=ps[:])
                outs.append(o)
            return outs

        def bcast_row(row_bf, width, tag="bc"):
            """[1,width] bf16 -> [P,width] f32 (SBUF) via outer product."""
            ps = ps_bc.tile([P, width], F32, name="bcast", tag=tag)
            nc.tensor.matmul(ps[:], lhsT=ones_row_b[:], rhs=row_bf[0:1, :width],
                             start=True, stop=True)
            sbt = scr.tile([P, width], F32, name="bcs", tag=f"bcs_{tag}",
                           bufs=2)
            nc.scalar.copy(out=sbt[:], in_=ps[:])
            return sbt

        def head_pieces(start, cnt):
            """Split global row range [start, start+cnt) at 128 boundaries.
            Yields (tile_idx, row_off, n)."""
            pieces = []
            while cnt > 0:
                t = start // P
                r = start % P
                take = min(P - r, cnt)
                pieces.append((t, r, take))
                start += take
                cnt -= take
            return pieces

        def head_rows(tiles, start, cnt, width):
            return [tiles[t][r:r + n, :width]
                    for (t, r, n) in head_pieces(start, cnt)]

        def mm_blocks(wname, KT, mt, rhs_tiles, Sw, ps, start=True, stop=True):
            """psum += W[:,mt-block].T @ rhs  (accumulate over KT k-blocks)."""
            for kt in range(KT):
                blk = wp.tile([P, P], BF16, name="wblk", tag="wblk")
                nc.sync.dma_start(out=blk[:], in_=dram[wname][kt, mt])
                nc.tensor.matmul(ps[:, :Sw], lhsT=blk[:],
                                 rhs=rhs_tiles[kt][:, :Sw],
                                 start=(start and kt == 0),
                                 stop=(stop and kt == KT - 1))

        def linear(wname, bname, KT, MT, rhs_tiles, Sw, evac, extra=None):
            """Generic y = W.T @ rhs + b; evac(mt, psum, bias_col)."""
            for mt in range(MT):
                ps = ps_mm.tile([P, Sw], F32, name="lin", tag="mm")
                mm_blocks(wname, KT, mt, rhs_tiles, Sw, ps)
                if extra is not None:
                    extra(mt, ps)
                bcol = vec_col(bname, mt) if bname else None
                evac(mt, ps, bcol)

        def layernorm(xin, xin_bf, gname, bname, d, Sw, out_tiles):
            """Transposed LN: stats via ones-matmul over bf16 copy."""
            DT = d // P
            ps_sum = ps_aux.tile([1, Sw], F32, name="lnsum", tag="aux")
            for t in range(DT):
                nc.tensor.matmul(ps_sum[:], lhsT=ones_kb[:],
                                 rhs=xin_bf[t][:, :Sw],
                                 start=(t == 0), stop=(t == DT - 1))
            ps_sq = ps_aux.tile([1, Sw], F32, name="lnsq", tag="aux")
            for t in range(DT):
                sq = scr.tile([P, Sw], BF16, name="sq", tag="sq", bufs=3)
                nc.scalar.activation(out=sq[:], in_=xin_bf[t][:, :Sw],
                                     func=AF.Square)
                nc.tensor.matmul(ps_sq[:], lhsT=ones_kb[:], rhs=sq[:],
                                 start=(t == 0), stop=(t == DT - 1))
            mean = rows.tile([1, Sw], F32, name="mean", tag="rows")
            nc.scalar.mul(out=mean[:], in_=ps_sum[:], mul=1.0 / d)
            msq = rows.tile([1, Sw], F32, name="msq", tag="rows")
            nc.scalar.mul(out=msq[:], in_=ps_sq[:], mul=1.0 / d)
            var = rows.tile([1, Sw], F32, name="var", tag="rows")
            m2 = rows.tile([1, Sw], F32, name="m2", tag="rows")
            nc.vector.tensor_tensor(out=m2[:], in0=mean[:], in1=mean[:],
                                    op=ALU.mult)
            nc.vector.tensor_tensor(out=var[:], in0=msq[:], in1=m2[:],
                                    op=ALU.subtract)
            std = rows.tile([1, Sw], F32, name="std", tag="rows")
            nc.scalar.activation(out=std[:], in_=var[:], func=AF.Sqrt,
                                 bias=1e-5)
            rstd = rows.tile([1, Sw], F32, name="rstd", tag="rows")
            nc.vector.reciprocal(out=rstd[:], in_=std[:])
            mrs = rows.tile([1, Sw], F32, name="mrs", tag="rows")
            nc.vector.tensor_tensor(out=mrs[:], in0=mean[:], in1=rstd[:],
                                    op=ALU.mult)
            rstd_b = rows.tile([1, Sw], BF16, name="rstdb", tag="rowsb")
            nc.scalar.copy(out=rstd_b[:], in_=rstd[:])
            mrs_b = rows.tile([1, Sw], BF16, name="mrsb", tag="rowsb")
            nc.scalar.copy(out=mrs_b[:], in_=mrs[:])
            rbc = bcast_row(rstd_b, Sw)
            mbc = bcast_row(mrs_b, Sw, tag="bc2")
            for t in range(DT):
                gcol = vec_col(gname, t)
                bcol = vec_col(bname, t)
                t1 = scr.tile([P, Sw], F32, name="ln_t1", tag="ln_t1", bufs=2)
                nc.vector.tensor_tensor(out=t1[:], in0=xin[t][:, :Sw],
                                        in1=rbc[:], op=ALU.mult)
                t2 = scr.tile([P, Sw], F32, name="ln_t2", tag="ln_t2", bufs=2)
                nc.vector.tensor_tensor(out=t2[:], in0=t1[:], in1=mbc[:],
                                        op=ALU.subtract)
                nc.vector.tensor_scalar(out=out_tiles[t][:, :Sw], in0=t2[:],
                                        scalar1=gcol[:], scalar2=bcol[:],
                                        op0=ALU.mult, op1=ALU.add)

        def attention(r, i, d, nh, qkv_w, qkv_b, Sw, masked):
            """Q/K head-padded attention. Returns oT tiles bf16
            [(nh*dhp)/P][P,S] in the PADDED layout (softmax-normalized);
            wo weights are row-padded to match."""
            DT = d // P
            dh = d // nh
            dhp = _dhp(nh, dh)
            QT = (nh * dhp) // P       # padded Q (and K) tile count
            n_k = (Sw + P - 1) // P
            qkvT = [scr.tile([P, S], BF16, name="qkvT", tag=f"qkvT{t}", bufs=1)
                    for t in range(2 * QT + DT)]
            for mt in range(2 * QT + DT):
                ps = ps_mm.tile([P, Sw], F32, name="qkv", tag="mm")
                mm_blocks(qkv_w, DT, mt, xln_bf, Sw, ps)
                bcol = vec_col(qkv_b, mt)
                nc.scalar.activation(out=qkvT[mt][:, :Sw], in_=ps[:],
                                     func=AF.Identity, bias=bcol[:])
                if Sw < S:
                    nc.vector.memset(qkvT[mt][:, Sw:S], 0.0)
            # V natural: transpose VT blocks
            vnat = [scr.tile([P, d], BF16, name="vnat", tag=f"vnat{k}", bufs=1)
                    for k in range(S // P)]
            for kt in range(n_k):
                for t in range(DT):
                    pst = ps_sc.tile([P, P], BF16, name="vtr", tag="sc")
                    nc.tensor.transpose(
                        out=pst[:],
                        in_=qkvT[2 * QT + t][:, kt * P:(kt + 1) * P],
                        identity=idn_b[:])
                    nc.scalar.copy(out=vnat[kt][:, t * P:(t + 1) * P],
                                   in_=pst[:])
            oT = [scr.tile([P, S], BF16, name="oT", tag=f"oT{t}", bufs=1)
                  for t in range(QT)]
            for t in range(QT):
                nc.vector.memset(oT[t][:], 0.0)  # padded rows must be finite
            for h in range(nh):
                kpieces = head_pieces(h * dhp, dhp)
                e_t = []
                for kt in range(n_k):
                    ps = ps_sc.tile([P, Sw], F32, name="sc", tag="sc")
                    for pi, (t, r0, n) in enumerate(kpieces):
                        nc.tensor.matmul(
                            ps[:],
                            lhsT=qkvT[QT + t][r0:r0 + n, kt * P:(kt + 1) * P],
                            rhs=qkvT[t][r0:r0 + n, :Sw], start=(pi == 0),
                            stop=(pi == len(kpieces) - 1))
                    e = scr.tile([P, Sw], BF16, name="e", tag="e", bufs=6)
                    nc.scalar.activation(out=e[:], in_=ps[:], func=AF.Exp,
                                         scale=1.0 / math.sqrt(dh))
                    if masked:
                        nc.vector.tensor_scalar(
                            out=e[:], in0=e[:], scalar1=vcol[r][kt][:],
                            scalar2=None, op0=ALU.mult)
                    e_t.append(e)
                ps_den = ps_aux.tile([1, Sw], F32, name="den", tag="aux")
                for kt in range(n_k):
                    nc.tensor.matmul(ps_den[:], lhsT=ones_kb[:], rhs=e_t[kt][:],
                                     start=(kt == 0), stop=(kt == n_k - 1))
                rrow = rows.tile([1, Sw], F32, name="rrow", tag="rows")
                nc.vector.reciprocal(out=rrow[:], in_=ps_den[:])
                rrow_b = rows.tile([1, Sw], BF16, name="rrowb", tag="rowsb")
                nc.scalar.copy(out=rrow_b[:], in_=rrow[:])
                rbc = bcast_row(rrow_b, Sw)
                # oT head = (V.T @ e) * recip. Output goes at h*dhp in the
                # padded layout; V columns are at h*dh (unpadded). Chunk so
                # output pieces stay within tiles at legal bases.
                for (t, r0, n_r) in head_pieces(h * dhp, dh):
                    if n_r <= 0:
                        continue
                    coff = (t * P + r0) - h * dhp  # offset within the head
                    if coff >= dh:
                        continue
                    n_r = min(n_r, dh - coff)
                    ps_av = ps_mm.tile([n_r, Sw], F32, name="av", tag="mm")
                    for kt in range(n_k):
                        nc.tensor.matmul(
                            ps_av[:],
                            lhsT=vnat[kt][:, h * dh + coff:h * dh + coff + n_r],
                            rhs=e_t[kt][:], start=(kt == 0),
                            stop=(kt == n_k - 1))
                    nc.vector.tensor_tensor(out=oT[t][r0:r0 + n_r, :Sw],
                                            in0=ps_av[:],
                                            in1=rbc[0:n_r, :],
                                            op=ALU.mult)
            return oT, QT

        # =========== per-row compute ===========
        def embed(r, idx_t):
            """Gather fused table rows, transpose, add pos enc."""
            for jt in range(S // P):
                xg = scr.tile([P, HD[0]], F32, name="xg", tag="xg", bufs=2)
                nc.gpsimd.indirect_dma_start(
                    out=xg[:], out_offset=None, in_=dram["table"][:],
                    in_offset=bass.IndirectOffsetOnAxis(
                        ap=idx_t[:, jt:jt + 1], axis=0))
                for dt in range(HD[0] // P):
                    pst = ps_sc.tile([P, P], F32, name="xtr", tag="sc")
                    nc.tensor.transpose(out=pst[:], in_=xg[:, dt * P:(dt + 1) * P],
                                        identity=idn_f[:])
                    posT_t = scr.tile([P, P], F32, name="pos", tag="pos", bufs=4)
                    nc.sync.dma_start(
                        out=posT_t[:],
                        in_=dram["posT"][dt * P:(dt + 1) * P, jt * P:(jt + 1) * P])
                    nc.vector.tensor_tensor(
                        out=xT[r][dt][:, jt * P:(jt + 1) * P], in0=pst[:],
                        in1=posT_t[:], op=ALU.add)
            for dt in range(HD[0] // P):
                nc.scalar.copy(out=xB[r][dt][:], in_=xT[r][dt][:])

        def enc_layer(r, i, d, Sw):
            DT = d // P
            global xln_bf
            # --- ln1 ---
            xln = [scr.tile([P, S], BF16, name="xln", tag=f"xln{t}", bufs=1)
                   for t in range(DT)]
            layernorm(xT[r], xB[r], f"L{i}_ln1g", f"L{i}_ln1b", d, Sw, xln)
            xln_bf = xln
            oT, QTe = attention(r, i, d, NH, f"L{i}_wqkv", f"L{i}_bqkv", Sw,
                                masked=(i > 0))
            # wo + residual -> h
            hT = [scr.tile([P, S], F32, name="hT", tag=f"hT{t}", bufs=1)
                  for t in range(DT)]
            hB = [scr.tile([P, S], BF16, name="hB", tag=f"hB{t}", bufs=1)
                  for t in range(DT)]

            def evac_wo(mt, ps, bcol):
                nc.vector.scalar_tensor_tensor(
                    out=hT[mt][:, :Sw], in0=ps[:], scalar=bcol[:],
                    in1=xT[r][mt][:, :Sw], op0=ALU.add, op1=ALU.add)
                nc.scalar.copy(out=hB[mt][:, :Sw], in_=hT[mt][:, :Sw])

            linear(f"L{i}_wo", f"L{i}_bo", QTe, DT, oT, Sw, evac_wo)
            # --- ln2 ---
            y2 = [scr.tile([P, S], BF16, name="y2", tag=f"xln{t}", bufs=1)
                  for t in range(DT)]
            layernorm(hT, hB, f"L{i}_ln2g", f"L{i}_ln2b", d, Sw, y2)
            xln_bf = y2
            # --- ff --- (gelu tiles alias dead qkvT/oT slots to save SBUF)
            def _gtag(j):
                return f"qkvT{j}" if j < 3 * DT else f"oT{j - 3 * DT}"
            gbf = [scr.tile([P, S], BF16, name="gbf", tag=_gtag(t), bufs=1)
                   for t in range(4 * DT)]

            def evac_gelu(mt, ps, bcol):
                nc.scalar.activation(out=gbf[mt][:, :Sw], in_=ps[:],
                                     func=AF.Gelu, bias=bcol[:])

            linear(f"L{i}_ff1", f"L{i}_fb1", DT, 4 * DT, xln_bf, Sw, evac_gelu)

            def evac_ff2(mt, ps, bcol):
                nc.vector.scalar_tensor_tensor(
                    out=hT[mt][:, :Sw], in0=ps[:], scalar=bcol[:],
                    in1=hT[mt][:, :Sw], op0=ALU.add, op1=ALU.add)
                nc.scalar.copy(out=hB[mt][:, :Sw], in_=hT[mt][:, :Sw])

            linear(f"L{i}_ff2", f"L{i}_fb2", 4 * DT, DT, gbf, Sw, evac_ff2)
            # --- final norm -> xB (bf16 only) ---
            layernorm(hT, hB, f"L{i}_normg", f"L{i}_normb", d, Sw,
                      [xB[r][t] for t in range(DT)])

        def merge_layer(r, i, d, Sw, idsf_row):
            """Gates, gid, merge attention mean, segment mean, combine."""
            DT = d // P
            n_k = (Sw + P - 1) // P
            global xln_bf
            xln_bf = xB[r]
            # ---- gates (probs unused at layer 0: decision comes from ids) ----
            if i > 0:
                gre = [scr.tile([P, S], BF16, name="gre", tag=f"oT{t}", bufs=1)
                       for t in range(DT)]
                for mt in range(DT):
                    ps = ps_mm.tile([P, Sw], F32, name="g1", tag="mm")
                    mm_blocks(f"L{i}_g1a", DT, mt,
                              [xB[r][t][:, 0:S] for t in range(DT)], Sw - 1, ps,
                              stop=False)
                    mm_blocks(f"L{i}_g1b", DT, mt,
                              [xB[r][t][:, 1:S] for t in range(DT)], Sw - 1, ps,
                              start=False)
                    bcol = vec_col(f"L{i}_bg1", mt)
                    nc.scalar.activation(out=gre[mt][:, :Sw - 1],
                                         in_=ps[:, :Sw - 1],
                                         func=AF.Relu, bias=bcol[:])
                ps_pr = ps_aux.tile([1, Sw - 1], F32, name="pr", tag="aux")
                for t in range(DT):
                    g2c = scr.tile([P, 1], BF16, name="g2c", tag="col2", bufs=8)
                    nc.sync.dma_start(out=g2c[:], in_=dram[f"L{i}_g2"][t])
                    nc.tensor.matmul(ps_pr[:], lhsT=g2c[:],
                                     rhs=gre[t][:, :Sw - 1],
                                     start=(t == 0), stop=(t == DT - 1))
            dec = rows.tile([1, S], F32, name="dec", tag="rows")
            if i == 0:
                # dec from ids: continuation bytes merge
                c1 = rows.tile([1, S], F32, name="c1", tag="rows")
                nc.vector.tensor_scalar(out=c1[:], in0=idsf_row[:],
                                        scalar1=127.5, scalar2=None,
                                        op0=ALU.is_gt)
                c2 = rows.tile([1, S], F32, name="c2", tag="rows")
                nc.vector.tensor_scalar(out=c2[:], in0=idsf_row[:],
                                        scalar1=191.5, scalar2=None,
                                        op0=ALU.is_lt)
                co = rows.tile([1, S], F32, name="co", tag="rows")
                nc.vector.tensor_tensor(out=co[:], in0=c1[:], in1=c2[:],
                                        op=ALU.mult)
                nc.vector.tensor_copy(out=dec[0:1, 0:Sw - 1],
                                      in_=co[0:1, 1:Sw])
            else:
                pr = rows.tile([1, Sw - 1], F32, name="prs", tag="rows")
                nc.scalar.activation(out=pr[:], in_=ps_pr[:], func=AF.Sigmoid,
                                     bias=scalars[i]["bg2"])
                nc.vector.tensor_scalar(out=dec[0:1, 0:Sw - 1], in0=pr[:],
                                        scalar1=scalars[i]["thr"],
                                        scalar2=None, op0=ALU.is_gt)
            # mask by valid[1:]
            decm = rows.tile([1, S], F32, name="decm", tag="rows")
            nc.vector.tensor_tensor(out=decm[0:1, 0:Sw - 1],
                                    in0=dec[0:1, 0:Sw - 1],
                                    in1=vrow[r][0:1, 1:Sw], op=ALU.mult)
            # m = 1 - dec (gid increments); mfull[0] = 0
            mfull = rows.tile([1, S], F32, name="mfull", tag="rows")
            nc.vector.memset(mfull[:], 0.0)
            nc.vector.tensor_scalar(out=mfull[0:1, 1:Sw],
                                    in0=decm[0:1, 0:Sw - 1], scalar1=-1.0,
                                    scalar2=1.0, op0=ALU.mult, op1=ALU.add)
            # n_groups = 1 + sum(m * valid[1:])
            mng = rows.tile([1, S], F32, name="mng", tag="rows")
            nc.vector.tensor_tensor(out=mng[0:1, 0:Sw - 1],
                                    in0=mfull[0:1, 1:Sw],
                                    in1=vrow[r][0:1, 1:Sw], op=ALU.mult)
            ng = rows.tile([1, 1], F32, name="ng", tag="ng", bufs=2)
            nc.vector.tensor_reduce(out=ng[:], in_=mng[0:1, 0:Sw - 1],
                                    axis=mybir.AxisListType.X, op=ALU.add)
            nc.vector.tensor_scalar(out=ng[:], in0=ng[:], scalar1=1.0,
                                    scalar2=None, op0=ALU.add)
            gid = rows.tile([1, S], F32, name="gid", tag="rows")
            nc.vector.memset(gid[:], 60000.0)  # tail never matches any group
            nc.vector.tensor_tensor_scan(
                out=gid[0:1, 0:Sw], data0=mfull[0:1, 0:Sw],
                data1=zeros_row[0:1, 0:Sw], initial=0.0,
                op0=ALU.add, op1=ALU.add)
            gcols = row_to_cols(gid, n_k)
            # MT[j, g] = (gid[j] == g), bf16
            SPW = SP
            MTt = [scr.tile([P, SPW], BF16, name="MT", tag=f"MT{k}", bufs=1)
                   for k in range(n_k)]
            for k in range(n_k):
                nc.vector.tensor_scalar(out=MTt[k][:], in0=iotab[:, :SPW],
                                        scalar1=gcols[k][:], scalar2=None,
                                        op0=ALU.is_equal)
            # counts + recip + new valid
            ps_cnt = ps_aux.tile([1, SPW], F32, name="cnt", tag="aux")
            for k in range(n_k):
                nc.tensor.matmul(ps_cnt[:], lhsT=ones_kb[:], rhs=MTt[k][:],
                                 start=(k == 0), stop=(k == n_k - 1))
            cclip = rows.tile([1, SPW], F32, name="cclip", tag="rows")
            nc.vector.tensor_scalar(out=cclip[:], in0=ps_cnt[:], scalar1=1.0,
                                    scalar2=None, op0=ALU.max)
            crec = rows.tile([1, SPW], F32, name="crec", tag="rows")
            nc.vector.reciprocal(out=crec[:], in_=cclip[:])
            nv = rows.tile([1, S], F32, name="nv", tag="nv", bufs=2)
            nc.vector.memset(nv[:], 0.0)
            nc.vector.tensor_scalar(out=nv[0:1, 0:SPW], in0=iotab[0:1, 0:SPW],
                                    scalar1=ng[:], scalar2=None, op0=ALU.is_lt)
            rv = rows.tile([1, SPW], F32, name="rv", tag="rows")
            nc.vector.tensor_tensor(out=rv[:], in0=crec[:],
                                    in1=nv[0:1, 0:SPW], op=ALU.mult)
            rvb = rows.tile([1, SPW], BF16, name="rvb", tag="rowsb")
            nc.scalar.copy(out=rvb[:], in_=rv[:])
            nvb = rows.tile([1, SPW], BF16, name="nvb", tag="rowsb")
            nc.scalar.copy(out=nvb[:], in_=nv[0:1, 0:SPW])
            # ---- merge attention (mean only) ----
            oTm, QTm = attention(r, i, d, MH, f"L{i}_mqkv", f"L{i}_mbqkv", Sw,
                                 masked=(i > 0))
            # masked mean over positions: am[dcol] = sum_j oT * vmask / Nv
            nv_cur = rows.tile([1, 1], F32, name="nv_cur", tag="ng", bufs=2)
            nc.vector.tensor_reduce(out=nv_cur[:], in_=vrow[r][0:1, 0:Sw],
                                    axis=mybir.AxisListType.X, op=ALU.add)
            nvrec = rows.tile([1, 1], F32, name="nvrec", tag="ng", bufs=2)
            nc.vector.reciprocal(out=nvrec[:], in_=nv_cur[:])
            nvrec_b = rows.tile([1, 1], BF16, name="nvrec_b", tag="ngb",
                                bufs=2)
            nc.scalar.copy(out=nvrec_b[:], in_=nvrec[:])
            ps_nb = ps_aux.tile([P, 1], F32, name="ps_nb", tag="aux")
            nc.tensor.matmul(ps_nb[:], lhsT=ones_row_b[:], rhs=nvrec_b[:],
                             start=True, stop=True)
            nvrec_c = scr.tile([P, 1], F32, name="nvrec_c", tag="col", bufs=8)
            nc.scalar.copy(out=nvrec_c[:], in_=ps_nb[:])
            vrow_b = rows.tile([1, S], BF16, name="vrowb", tag="rowsb")
            nc.scalar.copy(out=vrow_b[0:1, 0:Sw], in_=vrow[r][0:1, 0:Sw])
            vbc = bcast_row(vrow_b, Sw)
            om = []  # [QTm][P,1] bf16: mean of o over valid positions
            for t in range(QTm):
                tmp = scr.tile([P, Sw], F32, name="omtmp", tag="omtmp", bufs=2)
                nc.vector.tensor_tensor(out=tmp[:], in0=oTm[t][:, :Sw],
                                        in1=vbc[:], op=ALU.mult)
                o1 = scr.tile([P, 1], F32, name="om1", tag="col", bufs=8)
                nc.vector.tensor_reduce(out=o1[:], in_=tmp[:],
                                        axis=mybir.AxisListType.X, op=ALU.add)
                ob = scr.tile([P, 1], BF16, name="omb", tag="col2", bufs=8)
                nc.vector.tensor_scalar(out=ob[:], in0=o1[:],
                                        scalar1=nvrec_c[:], scalar2=0.1,
                                        op0=ALU.mult, op1=ALU.mult)
                om.append(ob)
            # am = mwo.T @ om + 0.1*mbo  -> [DT][P,1] f32
            am = []
            for mt in range(DT):
                ps = ps_aux.tile([P, 1], F32, name="am", tag="aux")
                for kt in range(QTm):
                    blk = wp.tile([P, P], BF16, name="wblk", tag="wblk")
                    nc.sync.dma_start(out=blk[:], in_=dram[f"L{i}_mwo"][kt, mt])
                    nc.tensor.matmul(ps[:], lhsT=blk[:], rhs=om[kt][:],
                                     start=(kt == 0), stop=(kt == DT - 1))
                bcol = vec_col(f"L{i}_mbo01", mt)
                a = scr.tile([P, 1], F32, name="amc", tag="col", bufs=8)
                nc.scalar.activation(out=a[:], in_=ps[:], func=AF.Identity,
                                     bias=bcol[:])
                am.append(a)
            # ---- segment mean + combine ----
            xnat = [scr.tile([P, d], BF16, name="xnat", tag=f"vnat{k}", bufs=1)
                    for k in range(n_k)]
            for k in range(n_k):
                for t in range(DT):
                    pst = ps_sc.tile([P, P], BF16, name="xtr2", tag="sc")
                    nc.tensor.transpose(out=pst[:],
                                        in_=xB[r][t][:, k * P:(k + 1) * P],
                                        identity=idn_b[:])
                    nc.scalar.copy(out=xnat[k][:, t * P:(t + 1) * P], in_=pst[:])
            rvbc = bcast_row(rvb, SPW)
            nvbc = bcast_row(nvb, SPW, tag="bc2")
            for dt in range(DT):
                ps = ps_mm.tile([P, SPW], F32, name="seg", tag="mm")
                for k in range(n_k):
                    nc.tensor.matmul(ps[:], lhsT=xnat[k][:, dt * P:(dt + 1) * P],
                                     rhs=MTt[k][:], start=(k == 0),
                                     stop=(k == n_k - 1))
                t1 = scr.tile([P, SPW], F32, name="cmb", tag="cmb", bufs=2)
                nc.vector.tensor_tensor(out=t1[:], in0=ps[:], in1=rvbc[:],
                                        op=ALU.mult)
                nc.vector.scalar_tensor_tensor(
                    out=xT[r][dt][:, 0:SPW], in0=t1[:], scalar=am[dt][:],
                    in1=nvbc[:], op0=ALU.add, op1=ALU.mult)
                nc.scalar.copy(out=xB[r][dt][:, 0:SPW],
                               in_=xT[r][dt][:, 0:SPW])
            # update valid state
            nc.vector.tensor_copy(out=vrow[r][:], in_=nv[:])
            nvcols = row_to_cols(nv, S // P)
            for k in range(S // P):
                nc.vector.tensor_copy(out=vcol[r][k][:], in_=nvcols[k][:])

        def proj_layer(r, i, d_in, d, Sw):
            """x = x @ proj + b (changes width d_in -> d)."""
            newT = [scr.tile([P, S], F32, name="pj", tag=f"hT{t}", bufs=1)
                    for t in range(d // P)]

            def evac(mt, ps, bcol):
                nc.scalar.activation(out=newT[mt][:, :Sw], in_=ps[:],
                                     func=AF.Identity, bias=bcol[:])

            linear(f"L{i}_proj", f"L{i}_projb", d_in // P, d // P,
                   xB[r], Sw, evac)
            for t in range(d // P):
                nc.vector.tensor_copy(out=xT[r][t][:, :Sw],
                                      in_=newT[t][:, :Sw])
                nc.scalar.copy(out=xB[r][t][:, :Sw], in_=newT[t][:, :Sw])

        def outputs(r, Sw):
            """Final x transpose + preds + DMA out."""
            # zero tails so outputs beyond SP are exact zeros
            for t in range(DTF):
                if Sw < S:
                    nc.vector.memset(xT[r][t][:, Sw:S], 0.0)
                    nc.vector.memset(xB[r][t][:, Sw:S], 0.0)
            # preds: [18, S] = wpred.T @ xB + b
            ps_p = ps_aux.tile([18, S], F32, name="pred", tag="aux")
            for t in range(DTF):
                wchunk = scr.tile([P, 18], BF16, name="wpr", tag="col2", bufs=8)
                nc.sync.dma_start(out=wchunk[:], in_=dram["wpred"][t])
                nc.tensor.matmul(ps_p[:], lhsT=wchunk[:], rhs=xB[r][t][:],
                                 start=(t == 0), stop=(t == DTF - 1))
            bpr = scr.tile([18, 1], F32, name="bpr", tag="col", bufs=8)
            nc.sync.dma_start(out=bpr[:], in_=dram["bpred"][:])
            predT = scr.tile([18, S], F32, name="predT", tag="predT", bufs=1)
            nc.scalar.activation(out=predT[:], in_=ps_p[:], func=AF.Identity,
                                 bias=bpr[:])
            for jt in range(S // P):
                # x natural out
                xno = scr.tile([P, dF], F32, name="xno", tag="xno", bufs=2)
                for t in range(DTF):
                    pst = ps_sc.tile([P, P], F32, name="xotr", tag="sc")
                    nc.tensor.transpose(out=pst[:],
                                        in_=xT[r][t][:, jt * P:(jt + 1) * P],
                                        identity=idn_f[:])
                    nc.scalar.copy(out=xno[:, t * P:(t + 1) * P], in_=pst[:])
                nc.sync.dma_start(out=out_x[r, jt * P:(jt + 1) * P, :],
                                  in_=xno[:])
                pst = ps_sc.tile([P, 18], F32, name="ptr", tag="sc")
                nc.tensor.matmul(pst[:], lhsT=predT[:, jt * P:(jt + 1) * P],
                                 rhs=idn_f[0:18, 0:18], start=True, stop=True)
                pno = scr.tile([P, 18], F32, name="pno", tag="pno", bufs=2)
                nc.scalar.copy(out=pno[:], in_=pst[:])
                nc.sync.dma_start(out=out_bl[r, jt * P:(jt + 1) * P, :],
                                  in_=pno[:, 0:4])
                nc.sync.dma_start(out=out_cl[r, jt * P:(jt + 1) * P, :],
                                  in_=pno[:, 4:18])
            nc.sync.dma_start(out=out_v[r:r + 1, :], in_=vrow[r][0:1, :])

        # ================= main program =================
        for r in range(RPC):
            idx_t = scr.tile([P, S // P], I32, name="idx", tag="idx", bufs=2)
            nc.sync.dma_start(out=idx_t[:], in_=ids_d[r])
            idsf_row = sb.tile([1, S], F32, name=f"idsf_{r}")
            nc.sync.dma_start(out=idsf_row[:], in_=idsf_d[r:r + 1, :])
            embed(r, idx_t)
            nc.vector.memset(vrow[r][:], 1.0)
            for k in range(S // P):
                nc.vector.memset(vcol[r][k][:], 1.0)
            Sw = S
            for i in range(n_layers):
                d_in = HD[i - 1] if i > 0 else HD[0]
                d = HD[i]
                if d_in != d:
                    proj_layer(r, i, d_in, d, Sw)
                enc_layer(r, i, d, Sw)
                merge_layer(r, i, d, Sw, idsf_row)
                Sw = SP
            outputs(r, Sw)

        for pl in (ps_bc, ps_aux, ps_sc, ps_mm, rows, wp, scr, sb):
            pl.release()

    return nc


def kernel(input_ids, params):
    import concourse.bass as bass  # noqa
    import concourse.mybir as mybir
    import bass_rust
    from concourse.bass_utils import run_bass_kernel_spmd

    consts, scalars, SP, ids = _prep_host(input_ids, params)
    n_layers = len(HD)
    nc = _build(SP, scalars, consts, n_layers=n_layers)
    legalize_waits(nc, mybir, bass_rust)

    # per-core inputs
    ids32 = ids.astype(np.int32)
    idssb = ids32.reshape(B, S // P, P).transpose(0, 2, 1)  # [B, P, S/P]
    idsf = ids32.astype(np.float32)
    in_maps = []
    for c in range(NCORES):
        m = dict(consts)
        m["ids"] = np.ascontiguousarray(idssb[c * RPC:(c + 1) * RPC])
        m["idsf"] = np.ascontiguousarray(idsf[c * RPC:(c + 1) * RPC])
        in_maps.append(m)
    import os
    import time as _time
    res = run_bass_kernel_spmd(nc, in_maps, list(range(NCORES))).results
    if os.environ.get("BASS_KERNEL_TIME"):
        # second run hits the warm jit/NEFF cache: wall time ~= dispatch +
        # input transfer + execute
        t0 = _time.time()
        res = run_bass_kernel_spmd(nc, in_maps, list(range(NCORES))).results
        print(f"warm rerun wall: {(_time.time() - t0) * 1e9:.0f} ns")

    dF = HD[n_layers - 1]
    x = np.zeros((B, S, dF), np.float32)
    bl = np.zeros((B, S, 4), np.float32)
    cl = np.zeros((B, S, 14), np.float32)
    vv = np.zeros((B, S), np.float32)
    for c in range(NCORES):
        x[c * RPC:(c + 1) * RPC] = res[c]["out_x"]
        bl[c * RPC:(c + 1) * RPC] = res[c]["out_bl"]
        cl[c * RPC:(c + 1) * RPC] = res[c]["out_cl"]
        vv[c * RPC:(c + 1) * RPC] = res[c]["out_v"]
    return x, bl, cl, vv


# revision 41
# speedup vs baseline: 1.4651x; 1.2828x over previous
"""BoundaryAwareEncoder Trainium2 kernel.

Data-parallel over batch: 16 rows -> 8 cores x 2 rows. Activations kept
TRANSPOSED in SBUF (xT [d, S], d on partitions) so every linear uses the
natural weight layout as lhsT. Softmax runs with keys on partitions (scores
magnitude <= ~1.5 so no max-subtraction needed). Segment-mean is a one-hot
matmul built on-device from a cumsum (tensor_tensor_scan) of the merge
decisions. bf16 matmuls with fp32 PSUM accumulation throughout.
"""
import math
import numpy as np

P = 128
HD = [512, 512, 640, 768, 768]
NH = 8
MH = 4
VOCAB = 260
B, S = 16, 512
NCORES = 8
RPC = B // NCORES  # rows per core


def _bf16(a):
    import ml_dtypes
    return np.asarray(a, dtype=np.float32).astype(ml_dtypes.bfloat16)


def _pos_enc(s, d):
    pos = np.arange(s, dtype=np.float32)[:, None]
    div = np.exp(np.arange(0, d, 2, dtype=np.float32) * (-math.log(10000.0) / d))
    pe = np.zeros((s, d), np.float32)
    pe[:, 0::2] = np.sin(pos * div)
    pe[:, 1::2] = np.cos(pos * div)
    return pe


def _np(x):
    return np.asarray(x)


def _blocks(w):
    """[K, M] -> [K/P, P, M] bf16 k-strips (M-contiguous per partition)."""
    K, M = w.shape
    return np.ascontiguousarray(_bf16(w).reshape(K // P, P, M))


def _chunks(v):
    """[d] -> [d/P, P, 1] f32."""
    v = np.asarray(v, np.float32)
    return np.ascontiguousarray(v.reshape(-1, P, 1))


def _dhp(nh, dh):
    """Smallest padded head size (mult of 32) whose per-head row pieces all
    start at partition 0/32/64 and with nh*dhp a multiple of 128."""
    legal = {0, 32, 64}
    cand = ((dh + 31) // 32) * 32
    while True:
        if (nh * cand) % P == 0:
            ok = True
            for h in range(nh):
                start = h * cand
                cnt = cand
                while cnt > 0:
                    r = start % P
                    take = min(P - r, cnt)
                    if r not in legal:
                        ok = False
                    start += take
                    cnt -= take
            if ok:
                return cand
        cand += 32


def _pad_qk(wqkv, bqkv, d, nh):
    """Column-pad each Q/K head to dhp columns (zeros); V unchanged."""
    dh = d // nh
    dhp = _dhp(nh, dh)
    wq, wk, wv = wqkv[:, :d], wqkv[:, d:2 * d], wqkv[:, 2 * d:]
    bq, bk, bv = bqkv[:d], bqkv[d:2 * d], bqkv[2 * d:]

    def padw(w):
        out = np.zeros((d, nh * dhp), np.float32)
        for h in range(nh):
            out[:, h * dhp:h * dhp + dh] = w[:, h * dh:(h + 1) * dh]
        return out

    def padb(b):
        out = np.zeros((nh * dhp,), np.float32)
        for h in range(nh):
            out[h * dhp:h * dhp + dh] = b[h * dh:(h + 1) * dh]
        return out

    wq2, wk2 = padw(wq), padw(wk)
    bq2, bk2 = padb(bq), padb(bk)
    return (np.concatenate([wq2, wk2, wv], axis=1),
            np.concatenate([bq2, bk2, bv]), dhp)


def _pad_wo(wo, d, nh):
    """Row-pad wo to the padded oT layout (zero rows for head padding)."""
    dh = d // nh
    dhp = _dhp(nh, dh)
    out = np.zeros((nh * dhp, d), np.float32)
    for h in range(nh):
        out[h * dhp:h * dhp + dh] = wo[h * dh:(h + 1) * dh]
    return out


def _vpack_layout(i, n_layers):
    """Ordered (key, nchunks) for layer i's packed per-partition vectors."""
    d = HD[i]
    d_in = HD[i - 1] if i > 0 else HD[0]
    DT = d // P
    dhp = _dhp(NH, d // NH)
    dhpm = _dhp(MH, d // MH)
    ents = []
    if d_in != d:
        ents.append(("projb", DT))
    ents += [("bqkv", (2 * NH * dhp + d) // P), ("bo", DT),
             ("ln1g", DT), ("ln1b", DT), ("ln2g", DT), ("ln2b", DT),
             ("normg", DT), ("normb", DT),
             ("fb1", 4 * DT), ("fb2", DT), ("bg1", DT),
             ("mbqkv", (2 * MH * dhpm + d) // P), ("mbo01", DT)]
    return ents


def legalize_waits(nc, mybir, bass_rust):
    """This walrus build accepts 1 embedded sem-wait per instruction (2 for
    EventSemaphore); Tile sometimes embeds more. Hoist extras onto inserted
    same-engine EventSemaphore instructions just before the offender."""
    SI = bass_rust.SyncInfo
    n_fixed = 0
    for fn in nc.m.functions:
        for bb in fn.blocks:
            insts = bb.instructions  # live list
            i = 0
            while i < len(insts):
                inst = insts[i]
                si = inst.sync_info
                waits = list(si.on_wait) if si and si.on_wait else []
                cap = 2 if isinstance(inst, mybir.InstEventSemaphore) else 1
                if len(waits) > cap:
                    extra, keep = waits[:-cap], waits[-cap:]
                    si.on_wait = keep
                    inst.sync_info = si
                    for j in range(0, len(extra), 2):
                        es = mybir.InstEventSemaphore(
                            name=f"I-wfix-{nc.next_id()}", ins=[], outs=[])
                        es.engine = inst.engine
                        es.sync_info = SI(on_wait=extra[j:j + 2], on_update=[])
                        insts.insert(i, es)
                        i += 1
                    n_fixed += 1
                i += 1
    return n_fixed


def _prep_host(input_ids, params):
    """Host-side parameter/layout prep. Returns (consts, SP)."""
    ids = _np(input_ids).astype(np.int64)
    p = params
    v = np.arange(VOCAB)
    cont_v = (v >= 128) & (v < 192)
    bnd_v = np.where(cont_v, 0, 1)
    cnt_v = (1 + (v >= 192) + (v >= 224) + (v >= 240)).astype(np.int64)
    emb = np.concatenate([
        _np(p["byte_emb"]),
        _np(p["boundary_emb"])[bnd_v],
        np.broadcast_to(_np(p["ctype_emb"])[0], (VOCAB, 128)),
        _np(p["count_emb"])[cnt_v],
        np.broadcast_to(_np(p["cpos_emb"])[0], (VOCAB, 128)),
    ], axis=1).astype(np.float32)
    table = emb @ _np(p["in_proj"]["w"]) + _np(p["in_proj"]["b"])  # [260, 512]
    posT = np.ascontiguousarray(_pos_enc(S, HD[0]).T)  # [512, 512]

    contm = (ids >= 128) & (ids < 192)
    ng0 = 1 + (~contm[:, 1:]).sum(1)
    SP = int(min(S, ((ng0.max() + 7) // 8) * 8))

    consts = {
        "table": table.astype(np.float32),
        "posT": posT.astype(np.float32),
        "iotab": np.broadcast_to(np.arange(S, dtype=np.float32)[None, :],
                                 (P, S)).copy(),
    }
    scalars = []  # per-layer baked floats (bg2, thr)
    n_layers = len(HD)
    for i, lp in enumerate(_np(params["layers"]) if False else params["layers"]):
        L, M = lp["layer"], lp["merge"]
        d = HD[i]
        pre = f"L{i}_"
        vecs = {}
        if L["proj"] is not None:
            consts[pre + "proj"] = _blocks(_np(L["proj"]["w"]))
            vecs["projb"] = _np(L["proj"]["b"])
        wq_, bq_, _ = _pad_qk(_np(L["attn"]["wqkv"]), _np(L["attn"]["bqkv"]),
                              d, NH)
        consts[pre + "wqkv"] = _blocks(wq_)
        vecs["bqkv"] = bq_
        consts[pre + "wo"] = _blocks(_pad_wo(_np(L["attn"]["wo"]), d, NH))
        vecs["bo"] = _np(L["attn"]["bo"])
        for nm in ("ln1", "ln2", "norm"):
            vecs[nm + "g"] = _np(L[nm]["g"])
            vecs[nm + "b"] = _np(L[nm]["b"])
        consts[pre + "ff1"] = _blocks(_np(L["ff1"]["w"]))
        vecs["fb1"] = _np(L["ff1"]["b"])
        consts[pre + "ff2"] = _blocks(_np(L["ff2"]["w"]))
        vecs["fb2"] = _np(L["ff2"]["b"])
        g1 = _np(M["gate1"]["w"])  # [2d, d]
        consts[pre + "g1a"] = _blocks(g1[:d])
        consts[pre + "g1b"] = _blocks(g1[d:])
        vecs["bg1"] = _np(M["gate1"]["b"])
        g2 = _np(M["gate2"]["w"])[:, 0]  # [d]
        consts[pre + "g2"] = np.ascontiguousarray(
            _bf16(g2).reshape(d // P, P, 1))
        mq_, mb_, _ = _pad_qk(_np(M["mattn"]["wqkv"]),
                              _np(M["mattn"]["bqkv"]), d, MH)
        consts[pre + "mqkv"] = _blocks(mq_)
        vecs["mbqkv"] = mb_
        consts[pre + "mwo"] = _blocks(_pad_wo(_np(M["mattn"]["wo"]), d, MH))
        vecs["mbo01"] = 0.1 * _np(M["mattn"]["bo"])
        cols = []
        for key, nch in _vpack_layout(i, n_layers):
            a = np.asarray(vecs[key], np.float32).reshape(nch, P).T  # [P, nch]
            cols.append(a)
        consts[pre + "vpack"] = np.ascontiguousarray(
            np.concatenate(cols, axis=1))
        scalars.append({
            "bg2": float(_np(M["gate2"]["b"])[0]),
            "thr": 0.7 + i / n_layers * 0.2,
        })
    wpred = np.concatenate(
        [_np(params["bound_pred"]["w"]), _np(params["ctype_pred"]["w"])], axis=1)
    bpred = np.concatenate(
        [_np(params["bound_pred"]["b"]), _np(params["ctype_pred"]["b"])])
    KT = wpred.shape[0] // P
    consts["wpred"] = np.ascontiguousarray(
        _bf16(wpred).reshape(KT, P, 18))
    consts["bpred"] = np.ascontiguousarray(bpred.astype(np.float32).reshape(18, 1))
    return consts, scalars, SP, ids


def _build(SP, scalars, consts_shapes, n_layers=5):
    import concourse.bass as bass
    import concourse.mybir as mybir
    import concourse.tile as tile
    from concourse.masks import make_identity

    F32 = mybir.dt.float32
    BF16 = mybir.dt.bfloat16
    I32 = mybir.dt.int32
    AF = mybir.ActivationFunctionType
    ALU = mybir.AluOpType

    nc = bass.Bass("TRN2")

    def reg_const(value, dtype=F32):
        key = (dtype, float(value))
        if key in nc.const_aps.aps:
            return
        t = nc.alloc_sbuf_tensor(f"const-{dtype.name}-{value}", [128, 1], dtype)
        nc.gpsimd.memset(t.ap(), float(value))
        nc.const_aps.aps[key] = t.ap()

    reg_const(1e-5)
    for sc in scalars:
        reg_const(sc["bg2"])
    nc.all_engine_barrier()

    # ---- DRAM I/O ----
    dram = {}
    for name, arr in consts_shapes.items():
        dt = {np.dtype(np.float32): F32, np.dtype(np.int32): I32}.get(
            np.dtype(arr.dtype), BF16)
        dram[name] = nc.dram_tensor(name, list(arr.shape), dt,
                                    kind="ExternalInput")
    ids_d = nc.dram_tensor("ids", [RPC, P, S // P], I32, kind="ExternalInput")
    idsf_d = nc.dram_tensor("idsf", [RPC, S], F32, kind="ExternalInput")
    dF = HD[n_layers - 1]
    out_x = nc.dram_tensor("out_x", [RPC, S, dF], F32, kind="ExternalOutput")
    out_bl = nc.dram_tensor("out_bl", [RPC, S, 4], F32, kind="ExternalOutput")
    out_cl = nc.dram_tensor("out_cl", [RPC, S, 14], F32, kind="ExternalOutput")
    out_v = nc.dram_tensor("out_v", [RPC, S], F32, kind="ExternalOutput")

    DTF = dF // P

    with tile.TileContext(nc) as tc:
        ctx_pools = []
        sb = tc.alloc_tile_pool(name="sb", bufs=1)           # persistent
        scr = tc.alloc_tile_pool(name="scr", bufs=3)         # scratch tiles
        wp = tc.alloc_tile_pool(name="wp", bufs=24)          # weight blocks
        rows = tc.alloc_tile_pool(name="rows", bufs=8)       # [1,S] rows
        ps_mm = tc.alloc_tile_pool(name="ps_mm", bufs=2, space="PSUM")
        ps_sc = tc.alloc_tile_pool(name="ps_sc", bufs=2, space="PSUM")
        ps_aux = tc.alloc_tile_pool(name="ps_aux", bufs=2, space="PSUM")
        ps_bc = tc.alloc_tile_pool(name="ps_bc", bufs=1, space="PSUM")

        # ---- constants ----
        idn_f = sb.tile([P, P], F32, name="idn_f")
        make_identity(nc, idn_f[:])
        idn_b = sb.tile([P, P], BF16, name="idn_b")
        make_identity(nc, idn_b[:])
        ones_kb = sb.tile([P, 1], BF16, name="ones_kb")
        nc.vector.memset(ones_kb[:], 1.0)
        ones_row_b = sb.tile([1, P], BF16, name="ones_row_b")
        nc.vector.memset(ones_row_b[:], 1.0)
        one_f = sb.tile([1, 1], F32, name="one_f")
        nc.vector.memset(one_f[:], 1.0)
        zeros_row = sb.tile([1, S], F32, name="zeros_row")
        nc.vector.memset(zeros_row[:], 0.0)
        iotab = sb.tile([P, S], F32, name="iotab")
        nc.sync.dma_start(out=iotab[:], in_=dram["iotab"][:])

        # ---- persistent per-row state ----
        xT = [[sb.tile([P, S], F32, name=f"xT_{r}_{t}") for t in range(DTF)]
              for r in range(RPC)]
        xB = [[sb.tile([P, S], BF16, name=f"xB_{r}_{t}") for t in range(DTF)]
              for r in range(RPC)]
        vrow = [sb.tile([1, S], F32, name=f"vrow_{r}") for r in range(RPC)]
        vcol = [[sb.tile([P, 1], F32, name=f"vcol_{r}_{k}") for k in range(S // P)]
                for r in range(RPC)]

        vp_state = {}

        def load_vpack(i):
            ents = _vpack_layout(i, n_layers)
            idx = {}
            off = 0
            for k, nch in ents:
                idx[k] = off
                off += nch
            vp_state["dram"] = dram[f"L{i}_vpack"]
            vp_state["idx"] = idx

        def vec_col(name, ct):
            """[P,1] bias column: slice of the layer vpack, loaded via DMA."""
            key = name.split("_", 1)[1]
            j = vp_state["idx"][key] + ct
            t = scr.tile([P, 1], F32, name="bias", tag="bias", bufs=12)
            nc.sync.dma_start(out=t[:],
                              in_=vp_state["dram"][:, j:j + 1])
            return t

        def row_to_cols(row, n_k, out_dtype=F32):
            """[1, n_k*P] row -> list of [P,1] columns (via K=1 matmuls)."""
            outs = []
            for k in range(n_k):
                ps = ps_aux.tile([P, 1], F32, name="r2c", tag="aux")
                nc.tensor.matmul(ps[:], lhsT=row[0:1, k * P:(k + 1) * P],
                                 rhs=one_f[:], start=True, stop=True)
                o = scr.tile([P, 1], out_dtype, name="col", tag="col", bufs=8)
                nc.vector.tensor_copy(out=o[:], in_=ps[:])
                outs.append(o)
            return outs

        def bcast_row(row_bf, width, tag="bc"):
            """[1,width] bf16 -> [P,width] f32 (SBUF) via outer product."""
            ps = ps_bc.tile([P, width], F32, name="bcast", tag=tag)
            nc.tensor.matmul(ps[:], lhsT=ones_row_b[:], rhs=row_bf[0:1, :width],
                             start=True, stop=True)
            sbt = scr.tile([P, width], F32, name="bcs", tag=f"bcs_{tag}",
                           bufs=2)
            nc.scalar.copy(out=sbt[:], in_=ps[:])
            return sbt

        def head_pieces(start, cnt):
            """Split global row range [start, start+cnt) at 128 boundaries.
            Yields (tile_idx, row_off, n)."""
            pieces = []
            while cnt > 0:
                t = start // P
                r = start % P
                take = min(P - r, cnt)
                pieces.append((t, r, take))
                start += take
                cnt -= take
            return pieces

        def head_rows(tiles, start, cnt, width):
            return [tiles[t][r:r + n, :width]
                    for (t, r, n) in head_pieces(start, cnt)]

        def mm_blocks(wname, KT, mt, rhs_tiles, Sw, ps, start=True, stop=True):
            """psum += W[:,mt-block].T @ rhs  (accumulate over KT k-strips)."""
            for kt in range(KT):
                blk = wp.tile([P, P], BF16, name="wblk1", tag="wblk1", bufs=16)
                nc.sync.dma_start(out=blk[:],
                                  in_=dram[wname][kt, :, mt * P:(mt + 1) * P])
                nc.tensor.matmul(ps[:, :Sw], lhsT=blk[:],
                                 rhs=rhs_tiles[kt][:, :Sw],
                                 start=(start and kt == 0),
                                 stop=(stop and kt == KT - 1))

        def linear(wname, bname, KT, MT, rhs_tiles, Sw, evac, extra=None):
            """y = W.T @ rhs + b; out-tiles in pairs so each weight DMA
            (one [P, 256] strip slice) feeds two matmuls."""
            for mt0 in range(0, MT, 2):
                mts = list(range(mt0, min(mt0 + 2, MT)))
                pss = [ps_mm.tile([P, Sw], F32, name="lin", tag="mm")
                       for _ in mts]
                for kt in range(KT):
                    w2 = wp.tile([P, 2 * P], BF16, name="wblk", tag="wblk")
                    nc.sync.dma_start(
                        out=w2[:, :len(mts) * P],
                        in_=dram[wname][kt, :,
                                            mt0 * P:(mt0 + len(mts)) * P])
                    for j in range(len(mts)):
                        nc.tensor.matmul(pss[j][:, :Sw],
                                         lhsT=w2[:, j * P:(j + 1) * P],
                                         rhs=rhs_tiles[kt][:, :Sw],
                                         start=(kt == 0),
                                         stop=(kt == KT - 1))
                for j, mt in enumerate(mts):
                    if extra is not None:
                        extra(mt, pss[j])
                    bcol = vec_col(bname, mt) if bname else None
                    evac(mt, pss[j], bcol)

        def layernorm(xin, xin_bf, gname, bname, d, Sw, out_tiles):
            """Transposed LN: stats via ones-matmul over bf16 copy."""
            DT = d // P
            ps_sum = ps_aux.tile([1, Sw], F32, name="lnsum", tag="aux")
            for t in range(DT):
                nc.tensor.matmul(ps_sum[:], lhsT=ones_kb[:],
                                 rhs=xin_bf[t][:, :Sw],
                                 start=(t == 0), stop=(t == DT - 1))
            ps_sq = ps_aux.tile([1, Sw], F32, name="lnsq", tag="aux")
            for t in range(DT):
                sq = scr.tile([P, Sw], BF16, name="sq", tag="sq", bufs=3)
                nc.scalar.activation(out=sq[:], in_=xin_bf[t][:, :Sw],
                                     func=AF.Square)
                nc.tensor.matmul(ps_sq[:], lhsT=ones_kb[:], rhs=sq[:],
                                 start=(t == 0), stop=(t == DT - 1))
            mean = rows.tile([1, Sw], F32, name="mean", tag="rows")
            nc.scalar.mul(out=mean[:], in_=ps_sum[:], mul=1.0 / d)
            msq = rows.tile([1, Sw], F32, name="msq", tag="rows")
            nc.scalar.mul(out=msq[:], in_=ps_sq[:], mul=1.0 / d)
            var = rows.tile([1, Sw], F32, name="var", tag="rows")
            m2 = rows.tile([1, Sw], F32, name="m2", tag="rows")
            nc.vector.tensor_tensor(out=m2[:], in0=mean[:], in1=mean[:],
                                    op=ALU.mult)
            nc.vector.tensor_tensor(out=var[:], in0=msq[:], in1=m2[:],
                                    op=ALU.subtract)
            std = rows.tile([1, Sw], F32, name="std", tag="rows")
            nc.scalar.activation(out=std[:], in_=var[:], func=AF.Sqrt,
                                 bias=1e-5)
            rstd = rows.tile([1, Sw], F32, name="rstd", tag="rows")
            nc.vector.reciprocal(out=rstd[:], in_=std[:])
            mrs = rows.tile([1, Sw], F32, name="mrs", tag="rows")
            nc.vector.tensor_tensor(out=mrs[:], in0=mean[:], in1=rstd[:],
                                    op=ALU.mult)
            rstd_b = rows.tile([1, Sw], BF16, name="rstdb", tag="rowsb")
            nc.scalar.copy(out=rstd_b[:], in_=rstd[:])
            mrs_b = rows.tile([1, Sw], BF16, name="mrsb", tag="rowsb")
            nc.scalar.copy(out=mrs_b[:], in_=mrs[:])
            rbc = bcast_row(rstd_b, Sw)
            mbc = bcast_row(mrs_b, Sw, tag="bc2")
            for t in range(DT):
                gcol = vec_col(gname, t)
                bcol = vec_col(bname, t)
                t1 = scr.tile([P, Sw], F32, name="ln_t1", tag="ln_t1", bufs=2)
                nc.vector.tensor_tensor(out=t1[:], in0=xin[t][:, :Sw],
                                        in1=rbc[:], op=ALU.mult)
                t2 = scr.tile([P, Sw], F32, name="ln_t2", tag="ln_t2", bufs=2)
                nc.vector.tensor_tensor(out=t2[:], in0=t1[:], in1=mbc[:],
                                        op=ALU.subtract)
                nc.vector.tensor_scalar(out=out_tiles[t][:, :Sw], in0=t2[:],
                                        scalar1=gcol[:], scalar2=bcol[:],
                                        op0=ALU.mult, op1=ALU.add)

        def attention(r, i, d, nh, qkv_w, qkv_b, Sw, masked):
            """Q/K head-padded attention. Returns oT tiles bf16
            [(nh*dhp)/P][P,S] in the PADDED layout (softmax-normalized);
            wo weights are row-padded to match."""
            DT = d // P
            dh = d // nh
            dhp = _dhp(nh, dh)
            QT = (nh * dhp) // P       # padded Q (and K) tile count
            n_k = (Sw + P - 1) // P
            qkvT = [scr.tile([P, S], BF16, name="qkvT", tag=f"qkvT{t}", bufs=1)
                    for t in range(2 * QT + DT)]

            def evac_qkv(mt, ps, bcol):
                nc.scalar.activation(out=qkvT[mt][:, :Sw], in_=ps[:],
                                     func=AF.Identity, bias=bcol[:])
                if Sw < S:
                    nc.vector.memset(qkvT[mt][:, Sw:S], 0.0)

            linear(qkv_w, qkv_b, DT, 2 * QT + DT, xln_bf, Sw, evac_qkv)
            # V natural: transpose VT blocks
            vnat = [scr.tile([P, d], BF16, name="vnat", tag=f"vnat{k}", bufs=1)
                    for k in range(S // P)]
            for kt in range(n_k):
                for t in range(DT):
                    pst = ps_sc.tile([P, P], BF16, name="vtr", tag="sc")
                    nc.tensor.transpose(
                        out=pst[:],
                        in_=qkvT[2 * QT + t][:, kt * P:(kt + 1) * P],
                        identity=idn_b[:])
                    nc.scalar.copy(out=vnat[kt][:, t * P:(t + 1) * P],
                                   in_=pst[:])
            oT = [scr.tile([P, S], BF16, name="oT", tag=f"oT{t}", bufs=1)
                  for t in range(QT)]
            for t in range(QT):
                nc.vector.memset(oT[t][:], 0.0)  # padded rows must be finite
            for h in range(nh):
                kpieces = head_pieces(h * dhp, dhp)
                e_t = []
                for kt in range(n_k):
                    ps = ps_sc.tile([P, Sw], F32, name="sc", tag="sc")
                    for pi, (t, r0, n) in enumerate(kpieces):
                        nc.tensor.matmul(
                            ps[:],
                            lhsT=qkvT[QT + t][r0:r0 + n, kt * P:(kt + 1) * P],
                            rhs=qkvT[t][r0:r0 + n, :Sw], start=(pi == 0),
                            stop=(pi == len(kpieces) - 1))
                    e = scr.tile([P, Sw], BF16, name="e", tag="e", bufs=6)
                    nc.scalar.activation(out=e[:], in_=ps[:], func=AF.Exp,
                                         scale=1.0 / math.sqrt(dh))
                    if masked:
                        nc.vector.tensor_scalar(
                            out=e[:], in0=e[:], scalar1=vcol[r][kt][:],
                            scalar2=None, op0=ALU.mult)
                    e_t.append(e)
                ps_den = ps_aux.tile([1, Sw], F32, name="den", tag="aux")
                for kt in range(n_k):
                    nc.tensor.matmul(ps_den[:], lhsT=ones_kb[:], rhs=e_t[kt][:],
                                     start=(kt == 0), stop=(kt == n_k - 1))
                rrow = rows.tile([1, Sw], F32, name="rrow", tag="rows")
                nc.vector.reciprocal(out=rrow[:], in_=ps_den[:])
                rrow_b = rows.tile([1, Sw], BF16, name="rrowb", tag="rowsb")
                nc.scalar.copy(out=rrow_b[:], in_=rrow[:])
                rbc = bcast_row(rrow_b, Sw)
                # oT head = (V.T @ e) * recip. Output goes at h*dhp in the
                # padded layout; V columns are at h*dh (unpadded). Chunk so
                # output pieces stay within tiles at legal bases.
                for (t, r0, n_r) in head_pieces(h * dhp, dh):
                    if n_r <= 0:
                        continue
                    coff = (t * P + r0) - h * dhp  # offset within the head
                    if coff >= dh:
                        continue
                    n_r = min(n_r, dh - coff)
                    ps_av = ps_mm.tile([n_r, Sw], F32, name="av", tag="mm")
                    for kt in range(n_k):
                        nc.tensor.matmul(
                            ps_av[:],
                            lhsT=vnat[kt][:, h * dh + coff:h * dh + coff + n_r],
                            rhs=e_t[kt][:], start=(kt == 0),
                            stop=(kt == n_k - 1))
                    nc.vector.tensor_tensor(out=oT[t][r0:r0 + n_r, :Sw],
                                            in0=ps_av[:],
                                            in1=rbc[0:n_r, :],
                                            op=ALU.mult)
            return oT, QT

        # =========== per-row compute ===========
        def embed(r, idx_t):
            """Gather fused table rows, transpose, add pos enc."""
            for jt in range(S // P):
                xg = scr.tile([P, HD[0]], F32, name="xg", tag="xg", bufs=2)
                nc.gpsimd.indirect_dma_start(
                    out=xg[:], out_offset=None, in_=dram["table"][:],
                    in_offset=bass.IndirectOffsetOnAxis(
                        ap=idx_t[:, jt:jt + 1], axis=0))
                for dt in range(HD[0] // P):
                    pst = ps_sc.tile([P, P], F32, name="xtr", tag="sc")
                    nc.tensor.transpose(out=pst[:], in_=xg[:, dt * P:(dt + 1) * P],
                                        identity=idn_f[:])
                    posT_t = scr.tile([P, P], F32, name="pos", tag="pos", bufs=4)
                    nc.sync.dma_start(
                        out=posT_t[:],
                        in_=dram["posT"][dt * P:(dt + 1) * P, jt * P:(jt + 1) * P])
                    nc.vector.tensor_tensor(
                        out=xT[r][dt][:, jt * P:(jt + 1) * P], in0=pst[:],
                        in1=posT_t[:], op=ALU.add)
            for dt in range(HD[0] // P):
                nc.scalar.copy(out=xB[r][dt][:], in_=xT[r][dt][:])

        def enc_layer(r, i, d, Sw):
            DT = d // P
            global xln_bf
            # --- ln1 ---
            xln = [scr.tile([P, S], BF16, name="xln", tag=f"xln{t}", bufs=1)
                   for t in range(DT)]
            layernorm(xT[r], xB[r], f"L{i}_ln1g", f"L{i}_ln1b", d, Sw, xln)
            xln_bf = xln
            oT, QTe = attention(r, i, d, NH, f"L{i}_wqkv", f"L{i}_bqkv", Sw,
                                masked=(i > 0))
            # wo + residual -> h
            hT = [scr.tile([P, S], F32, name="hT", tag=f"hT{t}", bufs=1)
                  for t in range(DT)]
            hB = [scr.tile([P, S], BF16, name="hB", tag=f"hB{t}", bufs=1)
                  for t in range(DT)]

            def evac_wo(mt, ps, bcol):
                nc.vector.scalar_tensor_tensor(
                    out=hT[mt][:, :Sw], in0=ps[:], scalar=bcol[:],
                    in1=xT[r][mt][:, :Sw], op0=ALU.add, op1=ALU.add)
                nc.scalar.copy(out=hB[mt][:, :Sw], in_=hT[mt][:, :Sw])

            linear(f"L{i}_wo", f"L{i}_bo", QTe, DT, oT, Sw, evac_wo)
            # --- ln2 ---
            y2 = [scr.tile([P, S], BF16, name="y2", tag=f"xln{t}", bufs=1)
                  for t in range(DT)]
            layernorm(hT, hB, f"L{i}_ln2g", f"L{i}_ln2b", d, Sw, y2)
            xln_bf = y2
            # --- ff --- (gelu tiles alias dead qkvT/oT slots to save SBUF)
            def _gtag(j):
                return f"qkvT{j}" if j < 3 * DT else f"oT{j - 3 * DT}"
            gbf = [scr.tile([P, S], BF16, name="gbf", tag=_gtag(t), bufs=1)
                   for t in range(4 * DT)]

            def evac_gelu(mt, ps, bcol):
                nc.scalar.activation(out=gbf[mt][:, :Sw], in_=ps[:],
                                     func=AF.Gelu, bias=bcol[:])

            linear(f"L{i}_ff1", f"L{i}_fb1", DT, 4 * DT, xln_bf, Sw, evac_gelu)

            def evac_ff2(mt, ps, bcol):
                nc.vector.scalar_tensor_tensor(
                    out=hT[mt][:, :Sw], in0=ps[:], scalar=bcol[:],
                    in1=hT[mt][:, :Sw], op0=ALU.add, op1=ALU.add)
                nc.scalar.copy(out=hB[mt][:, :Sw], in_=hT[mt][:, :Sw])

            linear(f"L{i}_ff2", f"L{i}_fb2", 4 * DT, DT, gbf, Sw, evac_ff2)
            # --- final norm -> xB (bf16 only) ---
            layernorm(hT, hB, f"L{i}_normg", f"L{i}_normb", d, Sw,
                      [xB[r][t] for t in range(DT)])

        def merge_layer(r, i, d, Sw, idsf_row):
            """Gates, gid, merge attention mean, segment mean, combine."""
            DT = d // P
            n_k = (Sw + P - 1) // P
            global xln_bf
            xln_bf = xB[r]
            # ---- gates (probs unused at layer 0: decision comes from ids) ----
            if i > 0:
                gre = [scr.tile([P, S], BF16, name="gre", tag=f"oT{t}", bufs=1)
                       for t in range(DT)]
                for mt in range(DT):
                    ps = ps_mm.tile([P, Sw], F32, name="g1", tag="mm")
                    mm_blocks(f"L{i}_g1a", DT, mt,
                              [xB[r][t][:, 0:S] for t in range(DT)], Sw - 1, ps,
                              stop=False)
                    mm_blocks(f"L{i}_g1b", DT, mt,
                              [xB[r][t][:, 1:S] for t in range(DT)], Sw - 1, ps,
                              start=False)
                    bcol = vec_col(f"L{i}_bg1", mt)
                    nc.scalar.activation(out=gre[mt][:, :Sw - 1],
                                         in_=ps[:, :Sw - 1],
                                         func=AF.Relu, bias=bcol[:])
                ps_pr = ps_aux.tile([1, Sw - 1], F32, name="pr", tag="aux")
                for t in range(DT):
                    g2c = scr.tile([P, 1], BF16, name="g2c", tag="col2", bufs=8)
                    nc.sync.dma_start(out=g2c[:], in_=dram[f"L{i}_g2"][t])
                    nc.tensor.matmul(ps_pr[:], lhsT=g2c[:],
                                     rhs=gre[t][:, :Sw - 1],
                                     start=(t == 0), stop=(t == DT - 1))
            dec = rows.tile([1, S], F32, name="dec", tag="rows")
            if i == 0:
                # dec from ids: continuation bytes merge
                c1 = rows.tile([1, S], F32, name="c1", tag="rows")
                nc.vector.tensor_scalar(out=c1[:], in0=idsf_row[:],
                                        scalar1=127.5, scalar2=None,
                                        op0=ALU.is_gt)
                c2 = rows.tile([1, S], F32, name="c2", tag="rows")
                nc.vector.tensor_scalar(out=c2[:], in0=idsf_row[:],
                                        scalar1=191.5, scalar2=None,
                                        op0=ALU.is_lt)
                co = rows.tile([1, S], F32, name="co", tag="rows")
                nc.vector.tensor_tensor(out=co[:], in0=c1[:], in1=c2[:],
                                        op=ALU.mult)
                nc.vector.tensor_copy(out=dec[0:1, 0:Sw - 1],
                                      in_=co[0:1, 1:Sw])
            else:
                pr = rows.tile([1, Sw - 1], F32, name="prs", tag="rows")
                nc.scalar.activation(out=pr[:], in_=ps_pr[:], func=AF.Sigmoid,
                                     bias=scalars[i]["bg2"])
                nc.vector.tensor_scalar(out=dec[0:1, 0:Sw - 1], in0=pr[:],
                                        scalar1=scalars[i]["thr"],
                                        scalar2=None, op0=ALU.is_gt)
            # mask by valid[1:]
            decm = rows.tile([1, S], F32, name="decm", tag="rows")
            nc.vector.tensor_tensor(out=decm[0:1, 0:Sw - 1],
                                    in0=dec[0:1, 0:Sw - 1],
                                    in1=vrow[r][0:1, 1:Sw], op=ALU.mult)
            # m = 1 - dec (gid increments); mfull[0] = 0
            mfull = rows.tile([1, S], F32, name="mfull", tag="rows")
            nc.vector.memset(mfull[:], 0.0)
            nc.vector.tensor_scalar(out=mfull[0:1, 1:Sw],
                                    in0=decm[0:1, 0:Sw - 1], scalar1=-1.0,
                                    scalar2=1.0, op0=ALU.mult, op1=ALU.add)
            # n_groups = 1 + sum(m * valid[1:])
            mng = rows.tile([1, S], F32, name="mng", tag="rows")
            nc.vector.tensor_tensor(out=mng[0:1, 0:Sw - 1],
                                    in0=mfull[0:1, 1:Sw],
                                    in1=vrow[r][0:1, 1:Sw], op=ALU.mult)
            ng = rows.tile([1, 1], F32, name="ng", tag="ng", bufs=2)
            nc.vector.tensor_reduce(out=ng[:], in_=mng[0:1, 0:Sw - 1],
                                    axis=mybir.AxisListType.X, op=ALU.add)
            nc.vector.tensor_scalar(out=ng[:], in0=ng[:], scalar1=1.0,
                                    scalar2=None, op0=ALU.add)
            gid = rows.tile([1, S], F32, name="gid", tag="rows")
            nc.vector.memset(gid[:], 60000.0)  # tail never matches any group
            nc.vector.tensor_tensor_scan(
                out=gid[0:1, 0:Sw], data0=mfull[0:1, 0:Sw],
                data1=zeros_row[0:1, 0:Sw], initial=0.0,
                op0=ALU.add, op1=ALU.add)
            gcols = row_to_cols(gid, n_k)
            # MT[j, g] = (gid[j] == g), bf16
            SPW = SP
            MTt = [scr.tile([P, SPW], BF16, name="MT", tag=f"MT{k}", bufs=1)
                   for k in range(n_k)]
            for k in range(n_k):
                nc.vector.tensor_scalar(out=MTt[k][:], in0=iotab[:, :SPW],
                                        scalar1=gcols[k][:], scalar2=None,
                                        op0=ALU.is_equal)
            # counts + recip + new valid
            ps_cnt = ps_aux.tile([1, SPW], F32, name="cnt", tag="aux")
            for k in range(n_k):
                nc.tensor.matmul(ps_cnt[:], lhsT=ones_kb[:], rhs=MTt[k][:],
                                 start=(k == 0), stop=(k == n_k - 1))
            cclip = rows.tile([1, SPW], F32, name="cclip", tag="rows")
            nc.vector.tensor_scalar(out=cclip[:], in0=ps_cnt[:], scalar1=1.0,
                                    scalar2=None, op0=ALU.max)
            crec = rows.tile([1, SPW], F32, name="crec", tag="rows")
            nc.vector.reciprocal(out=crec[:], in_=cclip[:])
            nv = rows.tile([1, S], F32, name="nv", tag="nv", bufs=2)
            nc.vector.memset(nv[:], 0.0)
            nc.vector.tensor_scalar(out=nv[0:1, 0:SPW], in0=iotab[0:1, 0:SPW],
                                    scalar1=ng[:], scalar2=None, op0=ALU.is_lt)
            rv = rows.tile([1, SPW], F32, name="rv", tag="rows")
            nc.vector.tensor_tensor(out=rv[:], in0=crec[:],
                                    in1=nv[0:1, 0:SPW], op=ALU.mult)
            rvb = rows.tile([1, SPW], BF16, name="rvb", tag="rowsb")
            nc.scalar.copy(out=rvb[:], in_=rv[:])
            nvb = rows.tile([1, SPW], BF16, name="nvb", tag="rowsb")
            nc.scalar.copy(out=nvb[:], in_=nv[0:1, 0:SPW])
            # ---- merge attention (mean only) ----
            oTm, QTm = attention(r, i, d, MH, f"L{i}_mqkv", f"L{i}_mbqkv", Sw,
                                 masked=(i > 0))
            # masked mean over positions: am[dcol] = sum_j oT * vmask / Nv
            nv_cur = rows.tile([1, 1], F32, name="nv_cur", tag="ng", bufs=2)
            nc.vector.tensor_reduce(out=nv_cur[:], in_=vrow[r][0:1, 0:Sw],
                                    axis=mybir.AxisListType.X, op=ALU.add)
            nvrec = rows.tile([1, 1], F32, name="nvrec", tag="ng", bufs=2)
            nc.vector.reciprocal(out=nvrec[:], in_=nv_cur[:])
            nvrec_b = rows.tile([1, 1], BF16, name="nvrec_b", tag="ngb",
                                bufs=2)
            nc.scalar.copy(out=nvrec_b[:], in_=nvrec[:])
            ps_nb = ps_aux.tile([P, 1], F32, name="ps_nb", tag="aux")
            nc.tensor.matmul(ps_nb[:], lhsT=ones_row_b[:], rhs=nvrec_b[:],
                             start=True, stop=True)
            nvrec_c = scr.tile([P, 1], F32, name="nvrec_c", tag="col", bufs=8)
            nc.scalar.copy(out=nvrec_c[:], in_=ps_nb[:])
            vrow_b = rows.tile([1, S], BF16, name="vrowb", tag="rowsb")
            nc.scalar.copy(out=vrow_b[0:1, 0:Sw], in_=vrow[r][0:1, 0:Sw])
            vbc = bcast_row(vrow_b, Sw)
            om = []  # [QTm][P,1] bf16: mean of o over valid positions
            for t in range(QTm):
                tmp = scr.tile([P, Sw], F32, name="omtmp", tag="omtmp", bufs=2)
                nc.vector.tensor_tensor(out=tmp[:], in0=oTm[t][:, :Sw],
                                        in1=vbc[:], op=ALU.mult)
                o1 = scr.tile([P, 1], F32, name="om1", tag="col", bufs=8)
                nc.vector.tensor_reduce(out=o1[:], in_=tmp[:],
                                        axis=mybir.AxisListType.X, op=ALU.add)
                ob = scr.tile([P, 1], BF16, name="omb", tag="col2", bufs=8)
                nc.vector.tensor_scalar(out=ob[:], in0=o1[:],
                                        scalar1=nvrec_c[:], scalar2=0.1,
                                        op0=ALU.mult, op1=ALU.mult)
                om.append(ob)
            # am = mwo.T @ om + 0.1*mbo  -> [DT][P,1] f32
            am = []
            for mt in range(DT):
                ps = ps_aux.tile([P, 1], F32, name="am", tag="aux")
                for kt in range(QTm):
                    blk = wp.tile([P, P], BF16, name="wblk1", tag="wblk1",
                                  bufs=16)
                    nc.sync.dma_start(
                        out=blk[:],
                        in_=dram[f"L{i}_mwo"][kt, :, mt * P:(mt + 1) * P])
                    nc.tensor.matmul(ps[:], lhsT=blk[:], rhs=om[kt][:],
                                     start=(kt == 0), stop=(kt == QTm - 1))
                bcol = vec_col(f"L{i}_mbo01", mt)
                a = scr.tile([P, 1], F32, name="amc", tag="col", bufs=8)
                nc.scalar.activation(out=a[:], in_=ps[:], func=AF.Identity,
                                     bias=bcol[:])
                am.append(a)
            # ---- segment mean + combine ----
            xnat = [scr.tile([P, d], BF16, name="xnat", tag=f"vnat{k}", bufs=1)
                    for k in range(n_k)]
            for k in range(n_k):
                for t in range(DT):
                    pst = ps_sc.tile([P, P], BF16, name="xtr2", tag="sc")
                    nc.tensor.transpose(out=pst[:],
                                        in_=xB[r][t][:, k * P:(k + 1) * P],
                                        identity=idn_b[:])
                    nc.scalar.copy(out=xnat[k][:, t * P:(t + 1) * P], in_=pst[:])
            rvbc = bcast_row(rvb, SPW)
            nvbc = bcast_row(nvb, SPW, tag="bc2")
            for dt in range(DT):
                ps = ps_mm.tile([P, SPW], F32, name="seg", tag="mm")
                for k in range(n_k):
                    nc.tensor.matmul(ps[:], lhsT=xnat[k][:, dt * P:(dt + 1) * P],
                                     rhs=MTt[k][:], start=(k == 0),
                                     stop=(k == n_k - 1))
                t1 = scr.tile([P, SPW], F32, name="cmb", tag="cmb", bufs=2)
                nc.vector.tensor_tensor(out=t1[:], in0=ps[:], in1=rvbc[:],
                                        op=ALU.mult)
                nc.vector.scalar_tensor_tensor(
                    out=xT[r][dt][:, 0:SPW], in0=t1[:], scalar=am[dt][:],
                    in1=nvbc[:], op0=ALU.add, op1=ALU.mult)
                nc.scalar.copy(out=xB[r][dt][:, 0:SPW],
                               in_=xT[r][dt][:, 0:SPW])
            # update valid state
            nc.vector.tensor_copy(out=vrow[r][:], in_=nv[:])
            nvcols = row_to_cols(nv, S // P)
            for k in range(S // P):
                nc.vector.tensor_copy(out=vcol[r][k][:], in_=nvcols[k][:])

        def proj_layer(r, i, d_in, d, Sw):
            """x = x @ proj + b (changes width d_in -> d)."""
            newT = [scr.tile([P, S], F32, name="pj", tag=f"hT{t}", bufs=1)
                    for t in range(d // P)]

            def evac(mt, ps, bcol):
                nc.scalar.activation(out=newT[mt][:, :Sw], in_=ps[:],
                                     func=AF.Identity, bias=bcol[:])

            linear(f"L{i}_proj", f"L{i}_projb", d_in // P, d // P,
                   xB[r], Sw, evac)
            for t in range(d // P):
                nc.vector.tensor_copy(out=xT[r][t][:, :Sw],
                                      in_=newT[t][:, :Sw])
                nc.scalar.copy(out=xB[r][t][:, :Sw], in_=newT[t][:, :Sw])

        def outputs(r, Sw):
            """Final x transpose + preds + DMA out."""
            # zero tails so outputs beyond SP are exact zeros
            for t in range(DTF):
                if Sw < S:
                    nc.vector.memset(xT[r][t][:, Sw:S], 0.0)
                    nc.vector.memset(xB[r][t][:, Sw:S], 0.0)
            # preds: [18, S] = wpred.T @ xB + b
            ps_p = ps_aux.tile([18, S], F32, name="pred", tag="aux")
            for t in range(DTF):
                wchunk = scr.tile([P, 18], BF16, name="wpr", tag="col2", bufs=8)
                nc.sync.dma_start(out=wchunk[:], in_=dram["wpred"][t])
                nc.tensor.matmul(ps_p[:], lhsT=wchunk[:], rhs=xB[r][t][:],
                                 start=(t == 0), stop=(t == DTF - 1))
            bpr = scr.tile([18, 1], F32, name="bpr", tag="col", bufs=8)
            nc.sync.dma_start(out=bpr[:], in_=dram["bpred"][:])
            predT = scr.tile([18, S], F32, name="predT", tag="predT", bufs=1)
            nc.scalar.activation(out=predT[:], in_=ps_p[:], func=AF.Identity,
                                 bias=bpr[:])
            for jt in range(S // P):
                # x natural out
                xno = scr.tile([P, dF], F32, name="xno", tag="xno", bufs=2)
                for t in range(DTF):
                    pst = ps_sc.tile([P, P], F32, name="xotr", tag="sc")
                    nc.tensor.transpose(out=pst[:],
                                        in_=xT[r][t][:, jt * P:(jt + 1) * P],
                                        identity=idn_f[:])
                    nc.scalar.copy(out=xno[:, t * P:(t + 1) * P], in_=pst[:])
                nc.sync.dma_start(out=out_x[r, jt * P:(jt + 1) * P, :],
                                  in_=xno[:])
                pst = ps_sc.tile([P, 18], F32, name="ptr", tag="sc")
                nc.tensor.matmul(pst[:], lhsT=predT[:, jt * P:(jt + 1) * P],
                                 rhs=idn_f[0:18, 0:18], start=True, stop=True)
                pno = scr.tile([P, 18], F32, name="pno", tag="pno", bufs=2)
                nc.scalar.copy(out=pno[:], in_=pst[:])
                nc.sync.dma_start(out=out_bl[r, jt * P:(jt + 1) * P, :],
                                  in_=pno[:, 0:4])
                nc.sync.dma_start(out=out_cl[r, jt * P:(jt + 1) * P, :],
                                  in_=pno[:, 4:18])
            nc.sync.dma_start(out=out_v[r:r + 1, :], in_=vrow[r][0:1, :])

        # ================= main program =================
        for r in range(RPC):
            idx_t = scr.tile([P, S // P], I32, name="idx", tag="idx", bufs=2)
            nc.sync.dma_start(out=idx_t[:], in_=ids_d[r])
            idsf_row = sb.tile([1, S], F32, name=f"idsf_{r}")
            nc.sync.dma_start(out=idsf_row[:], in_=idsf_d[r:r + 1, :])
            embed(r, idx_t)
            nc.vector.memset(vrow[r][:], 1.0)
            for k in range(S // P):
                nc.vector.memset(vcol[r][k][:], 1.0)
            Sw = S
            for i in range(n_layers):
                load_vpack(i)
                d_in = HD[i - 1] if i > 0 else HD[0]
                d = HD[i]
                if d_in != d:
                    proj_layer(r, i, d_in, d, Sw)
                enc_layer(r, i, d, Sw)
                merge_layer(r, i, d, Sw, idsf_row)
                Sw = SP
            outputs(r, Sw)

        for pl in (ps_bc, ps_aux, ps_sc, ps_mm, rows, wp, scr, sb):
            pl.release()

    return nc


def kernel(input_ids, params):
    import concourse.bass as bass  # noqa
    import concourse.mybir as mybir
    import bass_rust
    from concourse.bass_utils import run_bass_kernel_spmd

    consts, scalars, SP, ids = _prep_host(input_ids, params)
    n_layers = len(HD)
    nc = _build(SP, scalars, consts, n_layers=n_layers)
    legalize_waits(nc, mybir, bass_rust)

    # per-core inputs
    ids32 = ids.astype(np.int32)
    idssb = ids32.reshape(B, S // P, P).transpose(0, 2, 1)  # [B, P, S/P]
    idsf = ids32.astype(np.float32)
    in_maps = []
    for c in range(NCORES):
        m = dict(consts)
        m["ids"] = np.ascontiguousarray(idssb[c * RPC:(c + 1) * RPC])
        m["idsf"] = np.ascontiguousarray(idsf[c * RPC:(c + 1) * RPC])
        in_maps.append(m)
    import os
    import time as _time
    res = run_bass_kernel_spmd(nc, in_maps, list(range(NCORES))).results
    if os.environ.get("BASS_KERNEL_TIME"):
        # second run hits the warm jit/NEFF cache: wall time ~= dispatch +
        # input transfer + execute
        t0 = _time.time()
        res = run_bass_kernel_spmd(nc, in_maps, list(range(NCORES))).results
        print(f"warm rerun wall: {(_time.time() - t0) * 1e9:.0f} ns")

    dF = HD[n_layers - 1]
    x = np.zeros((B, S, dF), np.float32)
    bl = np.zeros((B, S, 4), np.float32)
    cl = np.zeros((B, S, 14), np.float32)
    vv = np.zeros((B, S), np.float32)
    for c in range(NCORES):
        x[c * RPC:(c + 1) * RPC] = res[c]["out_x"]
        bl[c * RPC:(c + 1) * RPC] = res[c]["out_bl"]
        cl[c * RPC:(c + 1) * RPC] = res[c]["out_cl"]
        vv[c * RPC:(c + 1) * RPC] = res[c]["out_v"]
    return x, bl, cl, vv


# revision 42
# speedup vs baseline: 1.5683x; 1.0705x over previous
"""BoundaryAwareEncoder Trainium2 kernel.

Data-parallel over batch: 16 rows -> 8 cores x 2 rows. Activations kept
TRANSPOSED in SBUF (xT [d, S], d on partitions) so every linear uses the
natural weight layout as lhsT. Softmax runs with keys on partitions (scores
magnitude <= ~1.5 so no max-subtraction needed). Segment-mean is a one-hot
matmul built on-device from a cumsum (tensor_tensor_scan) of the merge
decisions. bf16 matmuls with fp32 PSUM accumulation throughout.
"""
import math
import numpy as np

P = 128
HD = [512, 512, 640, 768, 768]
NH = 8
MH = 4
VOCAB = 260
B, S = 16, 512
NCORES = 8
RPC = B // NCORES  # rows per core


def _bf16(a):
    import ml_dtypes
    return np.asarray(a, dtype=np.float32).astype(ml_dtypes.bfloat16)


def _pos_enc(s, d):
    pos = np.arange(s, dtype=np.float32)[:, None]
    div = np.exp(np.arange(0, d, 2, dtype=np.float32) * (-math.log(10000.0) / d))
    pe = np.zeros((s, d), np.float32)
    pe[:, 0::2] = np.sin(pos * div)
    pe[:, 1::2] = np.cos(pos * div)
    return pe


def _np(x):
    return np.asarray(x)


def _blocks(w):
    """[K, M] -> [K/P, P, M] bf16 k-strips (M-contiguous per partition)."""
    K, M = w.shape
    return np.ascontiguousarray(_bf16(w).reshape(K // P, P, M))


def _chunks(v):
    """[d] -> [d/P, P, 1] f32."""
    v = np.asarray(v, np.float32)
    return np.ascontiguousarray(v.reshape(-1, P, 1))


def _dhp(nh, dh):
    """Smallest padded head size (mult of 32) whose per-head row pieces all
    start at partition 0/32/64 and with nh*dhp a multiple of 128."""
    legal = {0, 32, 64}
    cand = ((dh + 31) // 32) * 32
    while True:
        if (nh * cand) % P == 0:
            ok = True
            for h in range(nh):
                start = h * cand
                cnt = cand
                while cnt > 0:
                    r = start % P
                    take = min(P - r, cnt)
                    if r not in legal:
                        ok = False
                    start += take
                    cnt -= take
            if ok:
                return cand
        cand += 32


def _pad_qk(wqkv, bqkv, d, nh):
    """Column-pad each Q/K head to dhp columns (zeros); V unchanged."""
    dh = d // nh
    dhp = _dhp(nh, dh)
    wq, wk, wv = wqkv[:, :d], wqkv[:, d:2 * d], wqkv[:, 2 * d:]
    bq, bk, bv = bqkv[:d], bqkv[d:2 * d], bqkv[2 * d:]

    def padw(w):
        out = np.zeros((d, nh * dhp), np.float32)
        for h in range(nh):
            out[:, h * dhp:h * dhp + dh] = w[:, h * dh:(h + 1) * dh]
        return out

    def padb(b):
        out = np.zeros((nh * dhp,), np.float32)
        for h in range(nh):
            out[h * dhp:h * dhp + dh] = b[h * dh:(h + 1) * dh]
        return out

    wq2, wk2 = padw(wq), padw(wk)
    bq2, bk2 = padb(bq), padb(bk)
    return (np.concatenate([wq2, wk2, wv], axis=1),
            np.concatenate([bq2, bk2, bv]), dhp)


def _pad_wo(wo, d, nh):
    """Row-pad wo to the padded oT layout (zero rows for head padding)."""
    dh = d // nh
    dhp = _dhp(nh, dh)
    out = np.zeros((nh * dhp, d), np.float32)
    for h in range(nh):
        out[h * dhp:h * dhp + dh] = wo[h * dh:(h + 1) * dh]
    return out


def _vpack_layout(i, n_layers):
    """Ordered (key, nchunks) for layer i's packed per-partition vectors."""
    d = HD[i]
    d_in = HD[i - 1] if i > 0 else HD[0]
    DT = d // P
    dhp = _dhp(NH, d // NH)
    dhpm = _dhp(MH, d // MH)
    ents = []
    if d_in != d:
        ents.append(("projb", DT))
    ents += [("bqkv", (2 * NH * dhp + d) // P), ("bo", DT),
             ("ln1g", DT), ("ln1b", DT), ("ln2g", DT), ("ln2b", DT),
             ("normg", DT), ("normb", DT),
             ("fb1", 4 * DT), ("fb2", DT), ("bg1", DT),
             ("mbqkv", (2 * MH * dhpm + d) // P), ("mbo01", DT)]
    return ents


def legalize_waits(nc, mybir, bass_rust):
    """This walrus build accepts 1 embedded sem-wait per instruction (2 for
    EventSemaphore); Tile sometimes embeds more. Hoist extras onto inserted
    same-engine EventSemaphore instructions just before the offender."""
    SI = bass_rust.SyncInfo
    n_fixed = 0
    for fn in nc.m.functions:
        for bb in fn.blocks:
            insts = bb.instructions  # live list
            i = 0
            while i < len(insts):
                inst = insts[i]
                si = inst.sync_info
                waits = list(si.on_wait) if si and si.on_wait else []
                cap = 2 if isinstance(inst, mybir.InstEventSemaphore) else 1
                if len(waits) > cap:
                    extra, keep = waits[:-cap], waits[-cap:]
                    si.on_wait = keep
                    inst.sync_info = si
                    for j in range(0, len(extra), 2):
                        es = mybir.InstEventSemaphore(
                            name=f"I-wfix-{nc.next_id()}", ins=[], outs=[])
                        es.engine = inst.engine
                        es.sync_info = SI(on_wait=extra[j:j + 2], on_update=[])
                        insts.insert(i, es)
                        i += 1
                    n_fixed += 1
                i += 1
    return n_fixed


def _prep_host(input_ids, params):
    """Host-side parameter/layout prep. Returns (consts, SP)."""
    ids = _np(input_ids).astype(np.int64)
    p = params
    v = np.arange(VOCAB)
    cont_v = (v >= 128) & (v < 192)
    bnd_v = np.where(cont_v, 0, 1)
    cnt_v = (1 + (v >= 192) + (v >= 224) + (v >= 240)).astype(np.int64)
    emb = np.concatenate([
        _np(p["byte_emb"]),
        _np(p["boundary_emb"])[bnd_v],
        np.broadcast_to(_np(p["ctype_emb"])[0], (VOCAB, 128)),
        _np(p["count_emb"])[cnt_v],
        np.broadcast_to(_np(p["cpos_emb"])[0], (VOCAB, 128)),
    ], axis=1).astype(np.float32)
    table = emb @ _np(p["in_proj"]["w"]) + _np(p["in_proj"]["b"])  # [260, 512]
    posT = np.ascontiguousarray(_pos_enc(S, HD[0]).T)  # [512, 512]

    contm = (ids >= 128) & (ids < 192)
    ng0 = 1 + (~contm[:, 1:]).sum(1)
    SP = int(min(S, ((ng0.max() + 7) // 8) * 8))

    consts = {
        "table": table.astype(np.float32),
        "posT": posT.astype(np.float32),
        "iotab": np.broadcast_to(np.arange(S, dtype=np.float32)[None, :],
                                 (P, S)).copy(),
    }
    scalars = []  # per-layer baked floats (bg2, thr)
    n_layers = len(HD)
    for i, lp in enumerate(_np(params["layers"]) if False else params["layers"]):
        L, M = lp["layer"], lp["merge"]
        d = HD[i]
        pre = f"L{i}_"
        vecs = {}
        if L["proj"] is not None:
            consts[pre + "proj"] = _blocks(_np(L["proj"]["w"]))
            vecs["projb"] = _np(L["proj"]["b"])
        wq_, bq_, _ = _pad_qk(_np(L["attn"]["wqkv"]), _np(L["attn"]["bqkv"]),
                              d, NH)
        consts[pre + "wqkv"] = _blocks(wq_)
        vecs["bqkv"] = bq_
        consts[pre + "wo"] = _blocks(_pad_wo(_np(L["attn"]["wo"]), d, NH))
        vecs["bo"] = _np(L["attn"]["bo"])
        for nm in ("ln1", "ln2", "norm"):
            vecs[nm + "g"] = _np(L[nm]["g"])
            vecs[nm + "b"] = _np(L[nm]["b"])
        consts[pre + "ff1"] = _blocks(_np(L["ff1"]["w"]))
        vecs["fb1"] = _np(L["ff1"]["b"])
        consts[pre + "ff2"] = _blocks(_np(L["ff2"]["w"]))
        vecs["fb2"] = _np(L["ff2"]["b"])
        g1 = _np(M["gate1"]["w"])  # [2d, d]
        consts[pre + "g1a"] = _blocks(g1[:d])
        consts[pre + "g1b"] = _blocks(g1[d:])
        vecs["bg1"] = _np(M["gate1"]["b"])
        g2 = _np(M["gate2"]["w"])[:, 0]  # [d]
        consts[pre + "g2"] = np.ascontiguousarray(
            _bf16(g2).reshape(d // P, P, 1))
        mq_, mb_, _ = _pad_qk(_np(M["mattn"]["wqkv"]),
                              _np(M["mattn"]["bqkv"]), d, MH)
        consts[pre + "mqkv"] = _blocks(mq_)
        vecs["mbqkv"] = mb_
        consts[pre + "mwo"] = _blocks(_pad_wo(_np(M["mattn"]["wo"]), d, MH))
        vecs["mbo01"] = 0.1 * _np(M["mattn"]["bo"])
        cols = []
        for key, nch in _vpack_layout(i, n_layers):
            a = np.asarray(vecs[key], np.float32).reshape(nch, P).T  # [P, nch]
            cols.append(a)
        consts[pre + "vpack"] = np.ascontiguousarray(
            np.concatenate(cols, axis=1))
        scalars.append({
            "bg2": float(_np(M["gate2"]["b"])[0]),
            "thr": 0.7 + i / n_layers * 0.2,
        })
    wpred = np.concatenate(
        [_np(params["bound_pred"]["w"]), _np(params["ctype_pred"]["w"])], axis=1)
    bpred = np.concatenate(
        [_np(params["bound_pred"]["b"]), _np(params["ctype_pred"]["b"])])
    KT = wpred.shape[0] // P
    consts["wpred"] = np.ascontiguousarray(
        _bf16(wpred).reshape(KT, P, 18))
    consts["bpred"] = np.ascontiguousarray(bpred.astype(np.float32).reshape(18, 1))
    return consts, scalars, SP, ids


def _build(SP, scalars, consts_shapes, n_layers=5):
    import concourse.bass as bass
    import concourse.mybir as mybir
    import concourse.tile as tile
    from concourse.masks import make_identity

    F32 = mybir.dt.float32
    BF16 = mybir.dt.bfloat16
    I32 = mybir.dt.int32
    AF = mybir.ActivationFunctionType
    ALU = mybir.AluOpType

    nc = bass.Bass("TRN2")

    def reg_const(value, dtype=F32):
        key = (dtype, float(value))
        if key in nc.const_aps.aps:
            return
        t = nc.alloc_sbuf_tensor(f"const-{dtype.name}-{value}", [128, 1], dtype)
        nc.gpsimd.memset(t.ap(), float(value))
        nc.const_aps.aps[key] = t.ap()

    reg_const(1e-5)
    for sc in scalars:
        reg_const(sc["bg2"])
    nc.all_engine_barrier()

    # ---- DRAM I/O ----
    dram = {}
    for name, arr in consts_shapes.items():
        dt = {np.dtype(np.float32): F32, np.dtype(np.int32): I32}.get(
            np.dtype(arr.dtype), BF16)
        dram[name] = nc.dram_tensor(name, list(arr.shape), dt,
                                    kind="ExternalInput")
    ids_d = nc.dram_tensor("ids", [RPC, P, S // P], I32, kind="ExternalInput")
    idsf_d = nc.dram_tensor("idsf", [RPC, S], F32, kind="ExternalInput")
    dF = HD[n_layers - 1]
    out_x = nc.dram_tensor("out_x", [RPC, S, dF], F32, kind="ExternalOutput")
    out_bl = nc.dram_tensor("out_bl", [RPC, S, 4], F32, kind="ExternalOutput")
    out_cl = nc.dram_tensor("out_cl", [RPC, S, 14], F32, kind="ExternalOutput")
    out_v = nc.dram_tensor("out_v", [RPC, S], F32, kind="ExternalOutput")

    DTF = dF // P

    with tile.TileContext(nc) as tc:
        ctx_pools = []
        sb = tc.alloc_tile_pool(name="sb", bufs=1)           # persistent
        scr = tc.alloc_tile_pool(name="scr", bufs=3)         # scratch tiles
        wp = tc.alloc_tile_pool(name="wp", bufs=24)          # weight blocks
        rows = tc.alloc_tile_pool(name="rows", bufs=8)       # [1,S] rows
        ps_mm = tc.alloc_tile_pool(name="ps_mm", bufs=2, space="PSUM")
        ps_sc = tc.alloc_tile_pool(name="ps_sc", bufs=2, space="PSUM")
        ps_aux = tc.alloc_tile_pool(name="ps_aux", bufs=2, space="PSUM")
        ps_bc = tc.alloc_tile_pool(name="ps_bc", bufs=1, space="PSUM")

        # ---- constants ----
        idn_f = sb.tile([P, P], F32, name="idn_f")
        make_identity(nc, idn_f[:])
        idn_b = sb.tile([P, P], BF16, name="idn_b")
        make_identity(nc, idn_b[:])
        ones_kb = sb.tile([P, 1], BF16, name="ones_kb")
        nc.vector.memset(ones_kb[:], 1.0)
        ones_row_b = sb.tile([1, P], BF16, name="ones_row_b")
        nc.vector.memset(ones_row_b[:], 1.0)
        one_f = sb.tile([1, 1], F32, name="one_f")
        nc.vector.memset(one_f[:], 1.0)
        zeros_row = sb.tile([1, S], F32, name="zeros_row")
        nc.vector.memset(zeros_row[:], 0.0)
        iotab = sb.tile([P, S], F32, name="iotab")
        nc.sync.dma_start(out=iotab[:], in_=dram["iotab"][:])

        # ---- persistent per-row state ----
        xT = [[sb.tile([P, S], F32, name=f"xT_{r}_{t}") for t in range(DTF)]
              for r in range(RPC)]
        xB = [[sb.tile([P, S], BF16, name=f"xB_{r}_{t}") for t in range(DTF)]
              for r in range(RPC)]
        vrow = [sb.tile([1, S], F32, name=f"vrow_{r}") for r in range(RPC)]
        vcol = [[sb.tile([P, 1], F32, name=f"vcol_{r}_{k}") for k in range(S // P)]
                for r in range(RPC)]

        vp_state = {}

        def load_vpack(i):
            ents = _vpack_layout(i, n_layers)
            idx = {}
            off = 0
            for k, nch in ents:
                idx[k] = off
                off += nch
            t = scr.tile([P, off], F32, name="vpack", tag="vpack", bufs=2)
            nc.sync.dma_start(out=t[:], in_=dram[f"L{i}_vpack"][:])
            vp_state["tile"] = t
            vp_state["idx"] = idx

        def vec_col(name, ct):
            """[P,1] bias column: DVE-copied out of the layer's SBUF pack
            (ACT bias fetch needs a compact tile, not a strided slice)."""
            key = name.split("_", 1)[1]
            j = vp_state["idx"][key] + ct
            t = scr.tile([P, 1], F32, name="bias", tag="bias", bufs=12)
            nc.vector.tensor_copy(out=t[:],
                                  in_=vp_state["tile"][:, j:j + 1])
            return t

        def row_to_cols(row, n_k, out_dtype=F32):
            """[1, n_k*P] row -> list of [P,1] columns (via K=1 matmuls)."""
            outs = []
            for k in range(n_k):
                ps = ps_aux.tile([P, 1], F32, name="r2c", tag="aux")
                nc.tensor.matmul(ps[:], lhsT=row[0:1, k * P:(k + 1) * P],
                                 rhs=one_f[:], start=True, stop=True)
                o = scr.tile([P, 1], out_dtype, name="col", tag="col", bufs=8)
                nc.vector.tensor_copy(out=o[:], in_=ps[:])
                outs.append(o)
            return outs

        def bcast_row(row_bf, width, tag="bc"):
            """[1,width] bf16 -> [P,width] f32 (SBUF) via outer product."""
            ps = ps_bc.tile([P, width], F32, name="bcast", tag=tag)
            nc.tensor.matmul(ps[:], lhsT=ones_row_b[:], rhs=row_bf[0:1, :width],
                             start=True, stop=True)
            sbt = scr.tile([P, width], F32, name="bcs", tag=f"bcs_{tag}",
                           bufs=2)
            nc.scalar.copy(out=sbt[:], in_=ps[:])
            return sbt

        def head_pieces(start, cnt):
            """Split global row range [start, start+cnt) at 128 boundaries.
            Yields (tile_idx, row_off, n)."""
            pieces = []
            while cnt > 0:
                t = start // P
                r = start % P
                take = min(P - r, cnt)
                pieces.append((t, r, take))
                start += take
                cnt -= take
            return pieces

        def head_rows(tiles, start, cnt, width):
            return [tiles[t][r:r + n, :width]
                    for (t, r, n) in head_pieces(start, cnt)]

        def mm_blocks(wname, KT, mt, rhs_tiles, Sw, ps, start=True, stop=True):
            """psum += W[:,mt-block].T @ rhs  (accumulate over KT k-strips)."""
            for kt in range(KT):
                blk = wp.tile([P, P], BF16, name="wblk1", tag="wblk1", bufs=16)
                nc.sync.dma_start(out=blk[:],
                                  in_=dram[wname][kt, :, mt * P:(mt + 1) * P])
                nc.tensor.matmul(ps[:, :Sw], lhsT=blk[:],
                                 rhs=rhs_tiles[kt][:, :Sw],
                                 start=(start and kt == 0),
                                 stop=(stop and kt == KT - 1))

        def linear(wname, bname, KT, MT, rhs_tiles, Sw, evac, extra=None):
            """y = W.T @ rhs + b; out-tiles in pairs so each weight DMA
            (one [P, 256] strip slice) feeds two matmuls."""
            for mt0 in range(0, MT, 2):
                mts = list(range(mt0, min(mt0 + 2, MT)))
                pss = [ps_mm.tile([P, Sw], F32, name="lin", tag="mm")
                       for _ in mts]
                for kt in range(KT):
                    w2 = wp.tile([P, 2 * P], BF16, name="wblk", tag="wblk")
                    nc.sync.dma_start(
                        out=w2[:, :len(mts) * P],
                        in_=dram[wname][kt, :,
                                            mt0 * P:(mt0 + len(mts)) * P])
                    for j in range(len(mts)):
                        nc.tensor.matmul(pss[j][:, :Sw],
                                         lhsT=w2[:, j * P:(j + 1) * P],
                                         rhs=rhs_tiles[kt][:, :Sw],
                                         start=(kt == 0),
                                         stop=(kt == KT - 1))
                for j, mt in enumerate(mts):
                    if extra is not None:
                        extra(mt, pss[j])
                    bcol = vec_col(bname, mt) if bname else None
                    evac(mt, pss[j], bcol)

        def layernorm(xin, xin_bf, gname, bname, d, Sw, out_tiles):
            """Transposed LN: stats via ones-matmul over bf16 copy."""
            DT = d // P
            ps_sum = ps_aux.tile([1, Sw], F32, name="lnsum", tag="aux")
            for t in range(DT):
                nc.tensor.matmul(ps_sum[:], lhsT=ones_kb[:],
                                 rhs=xin_bf[t][:, :Sw],
                                 start=(t == 0), stop=(t == DT - 1))
            ps_sq = ps_aux.tile([1, Sw], F32, name="lnsq", tag="aux")
            for t in range(DT):
                sq = scr.tile([P, Sw], BF16, name="sq", tag="sq", bufs=3)
                nc.scalar.activation(out=sq[:], in_=xin_bf[t][:, :Sw],
                                     func=AF.Square)
                nc.tensor.matmul(ps_sq[:], lhsT=ones_kb[:], rhs=sq[:],
                                 start=(t == 0), stop=(t == DT - 1))
            mean = rows.tile([1, Sw], F32, name="mean", tag="rows")
            nc.scalar.mul(out=mean[:], in_=ps_sum[:], mul=1.0 / d)
            msq = rows.tile([1, Sw], F32, name="msq", tag="rows")
            nc.scalar.mul(out=msq[:], in_=ps_sq[:], mul=1.0 / d)
            var = rows.tile([1, Sw], F32, name="var", tag="rows")
            m2 = rows.tile([1, Sw], F32, name="m2", tag="rows")
            nc.vector.tensor_tensor(out=m2[:], in0=mean[:], in1=mean[:],
                                    op=ALU.mult)
            nc.vector.tensor_tensor(out=var[:], in0=msq[:], in1=m2[:],
                                    op=ALU.subtract)
            std = rows.tile([1, Sw], F32, name="std", tag="rows")
            nc.scalar.activation(out=std[:], in_=var[:], func=AF.Sqrt,
                                 bias=1e-5)
            rstd = rows.tile([1, Sw], F32, name="rstd", tag="rows")
            nc.vector.reciprocal(out=rstd[:], in_=std[:])
            mrs = rows.tile([1, Sw], F32, name="mrs", tag="rows")
            nc.vector.tensor_tensor(out=mrs[:], in0=mean[:], in1=rstd[:],
                                    op=ALU.mult)
            rstd_b = rows.tile([1, Sw], BF16, name="rstdb", tag="rowsb")
            nc.scalar.copy(out=rstd_b[:], in_=rstd[:])
            mrs_b = rows.tile([1, Sw], BF16, name="mrsb", tag="rowsb")
            nc.scalar.copy(out=mrs_b[:], in_=mrs[:])
            rbc = bcast_row(rstd_b, Sw)
            mbc = bcast_row(mrs_b, Sw, tag="bc2")
            for t in range(DT):
                gcol = vec_col(gname, t)
                bcol = vec_col(bname, t)
                t1 = scr.tile([P, Sw], F32, name="ln_t1", tag="ln_t1", bufs=2)
                nc.vector.tensor_tensor(out=t1[:], in0=xin[t][:, :Sw],
                                        in1=rbc[:], op=ALU.mult)
                t2 = scr.tile([P, Sw], F32, name="ln_t2", tag="ln_t2", bufs=2)
                nc.vector.tensor_tensor(out=t2[:], in0=t1[:], in1=mbc[:],
                                        op=ALU.subtract)
                nc.vector.tensor_scalar(out=out_tiles[t][:, :Sw], in0=t2[:],
                                        scalar1=gcol[:], scalar2=bcol[:],
                                        op0=ALU.mult, op1=ALU.add)

        def attention(r, i, d, nh, qkv_w, qkv_b, Sw, masked):
            """Q/K head-padded attention. Returns oT tiles bf16
            [(nh*dhp)/P][P,S] in the PADDED layout (softmax-normalized);
            wo weights are row-padded to match."""
            DT = d // P
            dh = d // nh
            dhp = _dhp(nh, dh)
            QT = (nh * dhp) // P       # padded Q (and K) tile count
            n_k = (Sw + P - 1) // P
            qkvT = [scr.tile([P, S], BF16, name="qkvT", tag=f"qkvT{t}", bufs=1)
                    for t in range(2 * QT + DT)]

            def evac_qkv(mt, ps, bcol):
                nc.scalar.activation(out=qkvT[mt][:, :Sw], in_=ps[:],
                                     func=AF.Identity, bias=bcol[:])
                if Sw < S:
                    nc.vector.memset(qkvT[mt][:, Sw:S], 0.0)

            linear(qkv_w, qkv_b, DT, 2 * QT + DT, xln_bf, Sw, evac_qkv)
            # V natural: transpose VT blocks
            vnat = [scr.tile([P, d], BF16, name="vnat", tag=f"vnat{k}", bufs=1)
                    for k in range(S // P)]
            for kt in range(n_k):
                for t in range(DT):
                    pst = ps_sc.tile([P, P], BF16, name="vtr", tag="sc")
                    nc.tensor.transpose(
                        out=pst[:],
                        in_=qkvT[2 * QT + t][:, kt * P:(kt + 1) * P],
                        identity=idn_b[:])
                    nc.scalar.copy(out=vnat[kt][:, t * P:(t + 1) * P],
                                   in_=pst[:])
            oT = [scr.tile([P, S], BF16, name="oT", tag=f"oT{t}", bufs=1)
                  for t in range(QT)]
            for t in range(QT):
                nc.vector.memset(oT[t][:], 0.0)  # padded rows must be finite
            for h in range(nh):
                kpieces = head_pieces(h * dhp, dhp)
                e_t = []
                for kt in range(n_k):
                    ps = ps_sc.tile([P, Sw], F32, name="sc", tag="sc")
                    for pi, (t, r0, n) in enumerate(kpieces):
                        nc.tensor.matmul(
                            ps[:],
                            lhsT=qkvT[QT + t][r0:r0 + n, kt * P:(kt + 1) * P],
                            rhs=qkvT[t][r0:r0 + n, :Sw], start=(pi == 0),
                            stop=(pi == len(kpieces) - 1))
                    e = scr.tile([P, Sw], BF16, name="e", tag="e", bufs=6)
                    nc.scalar.activation(out=e[:], in_=ps[:], func=AF.Exp,
                                         scale=1.0 / math.sqrt(dh))
                    if masked:
                        nc.vector.tensor_scalar(
                            out=e[:], in0=e[:], scalar1=vcol[r][kt][:],
                            scalar2=None, op0=ALU.mult)
                    e_t.append(e)
                ps_den = ps_aux.tile([1, Sw], F32, name="den", tag="aux")
                for kt in range(n_k):
                    nc.tensor.matmul(ps_den[:], lhsT=ones_kb[:], rhs=e_t[kt][:],
                                     start=(kt == 0), stop=(kt == n_k - 1))
                rrow = rows.tile([1, Sw], F32, name="rrow", tag="rows")
                nc.vector.reciprocal(out=rrow[:], in_=ps_den[:])
                rrow_b = rows.tile([1, Sw], BF16, name="rrowb", tag="rowsb")
                nc.scalar.copy(out=rrow_b[:], in_=rrow[:])
                rbc = bcast_row(rrow_b, Sw)
                # oT head = (V.T @ e) * recip. Output goes at h*dhp in the
                # padded layout; V columns are at h*dh (unpadded). Chunk so
                # output pieces stay within tiles at legal bases.
                for (t, r0, n_r) in head_pieces(h * dhp, dh):
                    if n_r <= 0:
                        continue
                    coff = (t * P + r0) - h * dhp  # offset within the head
                    if coff >= dh:
                        continue
                    n_r = min(n_r, dh - coff)
                    ps_av = ps_mm.tile([n_r, Sw], F32, name="av", tag="mm")
                    for kt in range(n_k):
                        nc.tensor.matmul(
                            ps_av[:],
                            lhsT=vnat[kt][:, h * dh + coff:h * dh + coff + n_r],
                            rhs=e_t[kt][:], start=(kt == 0),
                            stop=(kt == n_k - 1))
                    nc.vector.tensor_tensor(out=oT[t][r0:r0 + n_r, :Sw],
                                            in0=ps_av[:],
                                            in1=rbc[0:n_r, :],
                                            op=ALU.mult)
            return oT, QT

        # =========== per-row compute ===========
        def embed(r, idx_t):
            """Gather fused table rows, transpose, add pos enc."""
            for jt in range(S // P):
                xg = scr.tile([P, HD[0]], F32, name="xg", tag="xg", bufs=2)
                nc.gpsimd.indirect_dma_start(
                    out=xg[:], out_offset=None, in_=dram["table"][:],
                    in_offset=bass.IndirectOffsetOnAxis(
                        ap=idx_t[:, jt:jt + 1], axis=0))
                for dt in range(HD[0] // P):
                    pst = ps_sc.tile([P, P], F32, name="xtr", tag="sc")
                    nc.tensor.transpose(out=pst[:], in_=xg[:, dt * P:(dt + 1) * P],
                                        identity=idn_f[:])
                    posT_t = scr.tile([P, P], F32, name="pos", tag="pos", bufs=4)
                    nc.sync.dma_start(
                        out=posT_t[:],
                        in_=dram["posT"][dt * P:(dt + 1) * P, jt * P:(jt + 1) * P])
                    nc.vector.tensor_tensor(
                        out=xT[r][dt][:, jt * P:(jt + 1) * P], in0=pst[:],
                        in1=posT_t[:], op=ALU.add)
            for dt in range(HD[0] // P):
                nc.scalar.copy(out=xB[r][dt][:], in_=xT[r][dt][:])

        def enc_layer(r, i, d, Sw):
            DT = d // P
            global xln_bf
            # --- ln1 ---
            xln = [scr.tile([P, S], BF16, name="xln", tag=f"xln{t}", bufs=1)
                   for t in range(DT)]
            layernorm(xT[r], xB[r], f"L{i}_ln1g", f"L{i}_ln1b", d, Sw, xln)
            xln_bf = xln
            oT, QTe = attention(r, i, d, NH, f"L{i}_wqkv", f"L{i}_bqkv", Sw,
                                masked=(i > 0))
            # wo + residual -> h
            hT = [scr.tile([P, S], F32, name="hT", tag=f"hT{t}", bufs=1)
                  for t in range(DT)]
            hB = [scr.tile([P, S], BF16, name="hB", tag=f"hB{t}", bufs=1)
                  for t in range(DT)]

            def evac_wo(mt, ps, bcol):
                nc.vector.scalar_tensor_tensor(
                    out=hT[mt][:, :Sw], in0=ps[:], scalar=bcol[:],
                    in1=xT[r][mt][:, :Sw], op0=ALU.add, op1=ALU.add)
                nc.scalar.copy(out=hB[mt][:, :Sw], in_=hT[mt][:, :Sw])

            linear(f"L{i}_wo", f"L{i}_bo", QTe, DT, oT, Sw, evac_wo)
            # --- ln2 ---
            y2 = [scr.tile([P, S], BF16, name="y2", tag=f"xln{t}", bufs=1)
                  for t in range(DT)]
            layernorm(hT, hB, f"L{i}_ln2g", f"L{i}_ln2b", d, Sw, y2)
            xln_bf = y2
            # --- ff --- (gelu tiles alias dead qkvT/oT slots to save SBUF)
            def _gtag(j):
                return f"qkvT{j}" if j < 3 * DT else f"oT{j - 3 * DT}"
            gbf = [scr.tile([P, S], BF16, name="gbf", tag=_gtag(t), bufs=1)
                   for t in range(4 * DT)]

            def evac_gelu(mt, ps, bcol):
                nc.scalar.activation(out=gbf[mt][:, :Sw], in_=ps[:],
                                     func=AF.Gelu, bias=bcol[:])

            linear(f"L{i}_ff1", f"L{i}_fb1", DT, 4 * DT, xln_bf, Sw, evac_gelu)

            def evac_ff2(mt, ps, bcol):
                nc.vector.scalar_tensor_tensor(
                    out=hT[mt][:, :Sw], in0=ps[:], scalar=bcol[:],
                    in1=hT[mt][:, :Sw], op0=ALU.add, op1=ALU.add)
                nc.scalar.copy(out=hB[mt][:, :Sw], in_=hT[mt][:, :Sw])

            linear(f"L{i}_ff2", f"L{i}_fb2", 4 * DT, DT, gbf, Sw, evac_ff2)
            # --- final norm -> xB (bf16 only) ---
            layernorm(hT, hB, f"L{i}_normg", f"L{i}_normb", d, Sw,
                      [xB[r][t] for t in range(DT)])

        def merge_layer(r, i, d, Sw, idsf_row):
            """Gates, gid, merge attention mean, segment mean, combine."""
            DT = d // P
            n_k = (Sw + P - 1) // P
            global xln_bf
            xln_bf = xB[r]
            # ---- gates (probs unused at layer 0: decision comes from ids) ----
            if i > 0:
                gre = [scr.tile([P, S], BF16, name="gre", tag=f"oT{t}", bufs=1)
                       for t in range(DT)]
                rhs_a = [xB[r][t][:, 0:S] for t in range(DT)]
                rhs_b = [xB[r][t][:, 1:S] for t in range(DT)]
                for mt0 in range(0, DT, 2):
                    mts = list(range(mt0, min(mt0 + 2, DT)))
                    pss = [ps_mm.tile([P, Sw], F32, name="g1", tag="mm")
                           for _ in mts]
                    for wn, rhs_, st in ((f"L{i}_g1a", rhs_a, True),
                                         (f"L{i}_g1b", rhs_b, False)):
                        for kt in range(DT):
                            w2 = wp.tile([P, 2 * P], BF16, name="wblk",
                                         tag="wblk")
                            nc.sync.dma_start(
                                out=w2[:, :len(mts) * P],
                                in_=dram[wn][kt, :,
                                             mt0 * P:(mt0 + len(mts)) * P])
                            for j in range(len(mts)):
                                nc.tensor.matmul(
                                    pss[j][:, :Sw - 1],
                                    lhsT=w2[:, j * P:(j + 1) * P],
                                    rhs=rhs_[kt][:, :Sw - 1],
                                    start=(st and kt == 0),
                                    stop=(not st and kt == DT - 1))
                    for j, mt in enumerate(mts):
                        bcol = vec_col(f"L{i}_bg1", mt)
                        nc.scalar.activation(out=gre[mt][:, :Sw - 1],
                                             in_=pss[j][:, :Sw - 1],
                                             func=AF.Relu, bias=bcol[:])
                ps_pr = ps_aux.tile([1, Sw - 1], F32, name="pr", tag="aux")
                for t in range(DT):
                    g2c = scr.tile([P, 1], BF16, name="g2c", tag="col2", bufs=8)
                    nc.sync.dma_start(out=g2c[:], in_=dram[f"L{i}_g2"][t])
                    nc.tensor.matmul(ps_pr[:], lhsT=g2c[:],
                                     rhs=gre[t][:, :Sw - 1],
                                     start=(t == 0), stop=(t == DT - 1))
            dec = rows.tile([1, S], F32, name="dec", tag="rows")
            if i == 0:
                # dec from ids: continuation bytes merge
                c1 = rows.tile([1, S], F32, name="c1", tag="rows")
                nc.vector.tensor_scalar(out=c1[:], in0=idsf_row[:],
                                        scalar1=127.5, scalar2=None,
                                        op0=ALU.is_gt)
                c2 = rows.tile([1, S], F32, name="c2", tag="rows")
                nc.vector.tensor_scalar(out=c2[:], in0=idsf_row[:],
                                        scalar1=191.5, scalar2=None,
                                        op0=ALU.is_lt)
                co = rows.tile([1, S], F32, name="co", tag="rows")
                nc.vector.tensor_tensor(out=co[:], in0=c1[:], in1=c2[:],
                                        op=ALU.mult)
                nc.vector.tensor_copy(out=dec[0:1, 0:Sw - 1],
                                      in_=co[0:1, 1:Sw])
            else:
                pr = rows.tile([1, Sw - 1], F32, name="prs", tag="rows")
                nc.scalar.activation(out=pr[:], in_=ps_pr[:], func=AF.Sigmoid,
                                     bias=scalars[i]["bg2"])
                nc.vector.tensor_scalar(out=dec[0:1, 0:Sw - 1], in0=pr[:],
                                        scalar1=scalars[i]["thr"],
                                        scalar2=None, op0=ALU.is_gt)
            # mask by valid[1:]
            decm = rows.tile([1, S], F32, name="decm", tag="rows")
            nc.vector.tensor_tensor(out=decm[0:1, 0:Sw - 1],
                                    in0=dec[0:1, 0:Sw - 1],
                                    in1=vrow[r][0:1, 1:Sw], op=ALU.mult)
            # m = 1 - dec (gid increments); mfull[0] = 0
            mfull = rows.tile([1, S], F32, name="mfull", tag="rows")
            nc.vector.memset(mfull[:], 0.0)
            nc.vector.tensor_scalar(out=mfull[0:1, 1:Sw],
                                    in0=decm[0:1, 0:Sw - 1], scalar1=-1.0,
                                    scalar2=1.0, op0=ALU.mult, op1=ALU.add)
            # n_groups = 1 + sum(m * valid[1:])
            mng = rows.tile([1, S], F32, name="mng", tag="rows")
            nc.vector.tensor_tensor(out=mng[0:1, 0:Sw - 1],
                                    in0=mfull[0:1, 1:Sw],
                                    in1=vrow[r][0:1, 1:Sw], op=ALU.mult)
            ng = rows.tile([1, 1], F32, name="ng", tag="ng", bufs=2)
            nc.vector.tensor_reduce(out=ng[:], in_=mng[0:1, 0:Sw - 1],
                                    axis=mybir.AxisListType.X, op=ALU.add)
            nc.vector.tensor_scalar(out=ng[:], in0=ng[:], scalar1=1.0,
                                    scalar2=None, op0=ALU.add)
            gid = rows.tile([1, S], F32, name="gid", tag="rows")
            nc.vector.memset(gid[:], 60000.0)  # tail never matches any group
            nc.vector.tensor_tensor_scan(
                out=gid[0:1, 0:Sw], data0=mfull[0:1, 0:Sw],
                data1=zeros_row[0:1, 0:Sw], initial=0.0,
                op0=ALU.add, op1=ALU.add)
            gcols = row_to_cols(gid, n_k)
            # MT[j, g] = (gid[j] == g), bf16
            SPW = SP
            MTt = [scr.tile([P, SPW], BF16, name="MT", tag=f"MT{k}", bufs=1)
                   for k in range(n_k)]
            for k in range(n_k):
                nc.vector.tensor_scalar(out=MTt[k][:], in0=iotab[:, :SPW],
                                        scalar1=gcols[k][:], scalar2=None,
                                        op0=ALU.is_equal)
            # counts + recip + new valid
            ps_cnt = ps_aux.tile([1, SPW], F32, name="cnt", tag="aux")
            for k in range(n_k):
                nc.tensor.matmul(ps_cnt[:], lhsT=ones_kb[:], rhs=MTt[k][:],
                                 start=(k == 0), stop=(k == n_k - 1))
            cclip = rows.tile([1, SPW], F32, name="cclip", tag="rows")
            nc.vector.tensor_scalar(out=cclip[:], in0=ps_cnt[:], scalar1=1.0,
                                    scalar2=None, op0=ALU.max)
            crec = rows.tile([1, SPW], F32, name="crec", tag="rows")
            nc.vector.reciprocal(out=crec[:], in_=cclip[:])
            nv = rows.tile([1, S], F32, name="nv", tag="nv", bufs=2)
            nc.vector.memset(nv[:], 0.0)
            nc.vector.tensor_scalar(out=nv[0:1, 0:SPW], in0=iotab[0:1, 0:SPW],
                                    scalar1=ng[:], scalar2=None, op0=ALU.is_lt)
            rv = rows.tile([1, SPW], F32, name="rv", tag="rows")
            nc.vector.tensor_tensor(out=rv[:], in0=crec[:],
                                    in1=nv[0:1, 0:SPW], op=ALU.mult)
            rvb = rows.tile([1, SPW], BF16, name="rvb", tag="rowsb")
            nc.scalar.copy(out=rvb[:], in_=rv[:])
            nvb = rows.tile([1, SPW], BF16, name="nvb", tag="rowsb")
            nc.scalar.copy(out=nvb[:], in_=nv[0:1, 0:SPW])
            # ---- merge attention (mean only) ----
            oTm, QTm = attention(r, i, d, MH, f"L{i}_mqkv", f"L{i}_mbqkv", Sw,
                                 masked=(i > 0))
            # masked mean over positions: am[dcol] = sum_j oT * vmask / Nv
            nv_cur = rows.tile([1, 1], F32, name="nv_cur", tag="ng", bufs=2)
            nc.vector.tensor_reduce(out=nv_cur[:], in_=vrow[r][0:1, 0:Sw],
                                    axis=mybir.AxisListType.X, op=ALU.add)
            nvrec = rows.tile([1, 1], F32, name="nvrec", tag="ng", bufs=2)
            nc.vector.reciprocal(out=nvrec[:], in_=nv_cur[:])
            nvrec_b = rows.tile([1, 1], BF16, name="nvrec_b", tag="ngb",
                                bufs=2)
            nc.scalar.copy(out=nvrec_b[:], in_=nvrec[:])
            ps_nb = ps_aux.tile([P, 1], F32, name="ps_nb", tag="aux")
            nc.tensor.matmul(ps_nb[:], lhsT=ones_row_b[:], rhs=nvrec_b[:],
                             start=True, stop=True)
            nvrec_c = scr.tile([P, 1], F32, name="nvrec_c", tag="col", bufs=8)
            nc.scalar.copy(out=nvrec_c[:], in_=ps_nb[:])
            vrow_b = rows.tile([1, S], BF16, name="vrowb", tag="rowsb")
            nc.scalar.copy(out=vrow_b[0:1, 0:Sw], in_=vrow[r][0:1, 0:Sw])
            vbc = bcast_row(vrow_b, Sw)
            om = []  # [QTm][P,1] bf16: mean of o over valid positions
            for t in range(QTm):
                tmp = scr.tile([P, Sw], F32, name="omtmp", tag="omtmp", bufs=2)
                nc.vector.tensor_tensor(out=tmp[:], in0=oTm[t][:, :Sw],
                                        in1=vbc[:], op=ALU.mult)
                o1 = scr.tile([P, 1], F32, name="om1", tag="col", bufs=8)
                nc.vector.tensor_reduce(out=o1[:], in_=tmp[:],
                                        axis=mybir.AxisListType.X, op=ALU.add)
                ob = scr.tile([P, 1], BF16, name="omb", tag="col2", bufs=8)
                nc.vector.tensor_scalar(out=ob[:], in0=o1[:],
                                        scalar1=nvrec_c[:], scalar2=0.1,
                                        op0=ALU.mult, op1=ALU.mult)
                om.append(ob)
            # am = mwo.T @ om + 0.1*mbo  -> [DT][P,1] f32
            am = []
            for mt in range(DT):
                ps = ps_aux.tile([P, 1], F32, name="am", tag="aux")
                for kt in range(QTm):
                    blk = wp.tile([P, P], BF16, name="wblk1", tag="wblk1",
                                  bufs=16)
                    nc.sync.dma_start(
                        out=blk[:],
                        in_=dram[f"L{i}_mwo"][kt, :, mt * P:(mt + 1) * P])
                    nc.tensor.matmul(ps[:], lhsT=blk[:], rhs=om[kt][:],
                                     start=(kt == 0), stop=(kt == QTm - 1))
                bcol = vec_col(f"L{i}_mbo01", mt)
                a = scr.tile([P, 1], F32, name="amc", tag="col", bufs=8)
                nc.scalar.activation(out=a[:], in_=ps[:], func=AF.Identity,
                                     bias=bcol[:])
                am.append(a)
            # ---- segment mean + combine ----
            xnat = [scr.tile([P, d], BF16, name="xnat", tag=f"vnat{k}", bufs=1)
                    for k in range(n_k)]
            for k in range(n_k):
                for t in range(DT):
                    pst = ps_sc.tile([P, P], BF16, name="xtr2", tag="sc")
                    nc.tensor.transpose(out=pst[:],
                                        in_=xB[r][t][:, k * P:(k + 1) * P],
                                        identity=idn_b[:])
                    nc.scalar.copy(out=xnat[k][:, t * P:(t + 1) * P], in_=pst[:])
            rvbc = bcast_row(rvb, SPW)
            nvbc = bcast_row(nvb, SPW, tag="bc2")
            for dt in range(DT):
                ps = ps_mm.tile([P, SPW], F32, name="seg", tag="mm")
                for k in range(n_k):
                    nc.tensor.matmul(ps[:], lhsT=xnat[k][:, dt * P:(dt + 1) * P],
                                     rhs=MTt[k][:], start=(k == 0),
                                     stop=(k == n_k - 1))
                t1 = scr.tile([P, SPW], F32, name="cmb", tag="cmb", bufs=2)
                nc.vector.tensor_tensor(out=t1[:], in0=ps[:], in1=rvbc[:],
                                        op=ALU.mult)
                nc.vector.scalar_tensor_tensor(
                    out=xT[r][dt][:, 0:SPW], in0=t1[:], scalar=am[dt][:],
                    in1=nvbc[:], op0=ALU.add, op1=ALU.mult)
                nc.scalar.copy(out=xB[r][dt][:, 0:SPW],
                               in_=xT[r][dt][:, 0:SPW])
            # update valid state
            nc.vector.tensor_copy(out=vrow[r][:], in_=nv[:])
            nvcols = row_to_cols(nv, S // P)
            for k in range(S // P):
                nc.vector.tensor_copy(out=vcol[r][k][:], in_=nvcols[k][:])

        def proj_layer(r, i, d_in, d, Sw):
            """x = x @ proj + b (changes width d_in -> d)."""
            newT = [scr.tile([P, S], F32, name="pj", tag=f"hT{t}", bufs=1)
                    for t in range(d // P)]

            def evac(mt, ps, bcol):
                nc.scalar.activation(out=newT[mt][:, :Sw], in_=ps[:],
                                     func=AF.Identity, bias=bcol[:])

            linear(f"L{i}_proj", f"L{i}_projb", d_in // P, d // P,
                   xB[r], Sw, evac)
            for t in range(d // P):
                nc.vector.tensor_copy(out=xT[r][t][:, :Sw],
                                      in_=newT[t][:, :Sw])
                nc.scalar.copy(out=xB[r][t][:, :Sw], in_=newT[t][:, :Sw])

        def outputs(r, Sw):
            """Final x transpose + preds + DMA out."""
            # zero tails so outputs beyond SP are exact zeros
            for t in range(DTF):
                if Sw < S:
                    nc.vector.memset(xT[r][t][:, Sw:S], 0.0)
                    nc.vector.memset(xB[r][t][:, Sw:S], 0.0)
            # preds: [18, S] = wpred.T @ xB + b
            ps_p = ps_aux.tile([18, S], F32, name="pred", tag="aux")
            for t in range(DTF):
                wchunk = scr.tile([P, 18], BF16, name="wpr", tag="col2", bufs=8)
                nc.sync.dma_start(out=wchunk[:], in_=dram["wpred"][t])
                nc.tensor.matmul(ps_p[:], lhsT=wchunk[:], rhs=xB[r][t][:],
                                 start=(t == 0), stop=(t == DTF - 1))
            bpr = scr.tile([18, 1], F32, name="bpr", tag="col", bufs=8)
            nc.sync.dma_start(out=bpr[:], in_=dram["bpred"][:])
            predT = scr.tile([18, S], F32, name="predT", tag="predT", bufs=1)
            nc.scalar.activation(out=predT[:], in_=ps_p[:], func=AF.Identity,
                                 bias=bpr[:])
            for jt in range(S // P):
                # x natural out
                xno = scr.tile([P, dF], F32, name="xno", tag="xno", bufs=2)
                for t in range(DTF):
                    pst = ps_sc.tile([P, P], F32, name="xotr", tag="sc")
                    nc.tensor.transpose(out=pst[:],
                                        in_=xT[r][t][:, jt * P:(jt + 1) * P],
                                        identity=idn_f[:])
                    nc.scalar.copy(out=xno[:, t * P:(t + 1) * P], in_=pst[:])
                nc.sync.dma_start(out=out_x[r, jt * P:(jt + 1) * P, :],
                                  in_=xno[:])
                pst = ps_sc.tile([P, 18], F32, name="ptr", tag="sc")
                nc.tensor.matmul(pst[:], lhsT=predT[:, jt * P:(jt + 1) * P],
                                 rhs=idn_f[0:18, 0:18], start=True, stop=True)
                pno = scr.tile([P, 18], F32, name="pno", tag="pno", bufs=2)
                nc.scalar.copy(out=pno[:], in_=pst[:])
                nc.sync.dma_start(out=out_bl[r, jt * P:(jt + 1) * P, :],
                                  in_=pno[:, 0:4])
                nc.sync.dma_start(out=out_cl[r, jt * P:(jt + 1) * P, :],
                                  in_=pno[:, 4:18])
            nc.sync.dma_start(out=out_v[r:r + 1, :], in_=vrow[r][0:1, :])

        # ================= main program =================
        for r in range(RPC):
            idx_t = scr.tile([P, S // P], I32, name="idx", tag="idx", bufs=2)
            nc.sync.dma_start(out=idx_t[:], in_=ids_d[r])
            idsf_row = sb.tile([1, S], F32, name=f"idsf_{r}")
            nc.sync.dma_start(out=idsf_row[:], in_=idsf_d[r:r + 1, :])
            embed(r, idx_t)
            nc.vector.memset(vrow[r][:], 1.0)
            for k in range(S // P):
                nc.vector.memset(vcol[r][k][:], 1.0)
            Sw = S
            for i in range(n_layers):
                load_vpack(i)
                d_in = HD[i - 1] if i > 0 else HD[0]
                d = HD[i]
                if d_in != d:
                    proj_layer(r, i, d_in, d, Sw)
                enc_layer(r, i, d, Sw)
                merge_layer(r, i, d, Sw, idsf_row)
                Sw = SP
            outputs(r, Sw)

        for pl in (ps_bc, ps_aux, ps_sc, ps_mm, rows, wp, scr, sb):
            pl.release()

    return nc


def kernel(input_ids, params):
    import concourse.bass as bass  # noqa
    import concourse.mybir as mybir
    import bass_rust
    from concourse.bass_utils import run_bass_kernel_spmd

    consts, scalars, SP, ids = _prep_host(input_ids, params)
    n_layers = len(HD)
    nc = _build(SP, scalars, consts, n_layers=n_layers)
    legalize_waits(nc, mybir, bass_rust)

    # per-core inputs
    ids32 = ids.astype(np.int32)
    idssb = ids32.reshape(B, S // P, P).transpose(0, 2, 1)  # [B, P, S/P]
    idsf = ids32.astype(np.float32)
    in_maps = []
    for c in range(NCORES):
        m = dict(consts)
        m["ids"] = np.ascontiguousarray(idssb[c * RPC:(c + 1) * RPC])
        m["idsf"] = np.ascontiguousarray(idsf[c * RPC:(c + 1) * RPC])
        in_maps.append(m)
    import os
    import time as _time
    res = run_bass_kernel_spmd(nc, in_maps, list(range(NCORES))).results
    if os.environ.get("BASS_KERNEL_TIME"):
        # second run hits the warm jit/NEFF cache: wall time ~= dispatch +
        # input transfer + execute
        t0 = _time.time()
        res = run_bass_kernel_spmd(nc, in_maps, list(range(NCORES))).results
        print(f"warm rerun wall: {(_time.time() - t0) * 1e9:.0f} ns")

    dF = HD[n_layers - 1]
    x = np.zeros((B, S, dF), np.float32)
    bl = np.zeros((B, S, 4), np.float32)
    cl = np.zeros((B, S, 14), np.float32)
    vv = np.zeros((B, S), np.float32)
    for c in range(NCORES):
        x[c * RPC:(c + 1) * RPC] = res[c]["out_x"]
        bl[c * RPC:(c + 1) * RPC] = res[c]["out_bl"]
        cl[c * RPC:(c + 1) * RPC] = res[c]["out_cl"]
        vv[c * RPC:(c + 1) * RPC] = res[c]["out_v"]
    return x, bl, cl, vv
